# revision 51
# baseline (speedup 1.0000x reference)
"""Trainium2 Bass kernel for the AudNet 4-layer LIF spiking network.

Reference computation (per time step t of 81, batch 4096):
    s1, m1 = lif(x_t @ w1.T + b1, m1)     # 129 -> 1000
    s2, m2 = lif(s1 @ w2.T + b2, m2)      # 1000 -> 1000
    s3, m3 = lif(s2 @ w3.T + b3, m3)      # 1000 -> 20
    s4, m4 = lif(s3 @ w4.T + b4, m4)      # 20 -> 10
with lif: reset = (m > 1); m' = 0.95*m + cur - reset; spk = (m' > 1)
Outputs: (s4, m4) per step -> each [81, 4096, 10].

Strategy:
- Data parallel over 8 NeuronCores: 512 batch rows per core; weights
  replicated; no cross-device traffic.
- Hidden-on-partition, batch-on-free layout: weights are the stationary
  lhsT, spikes/x the moving rhs; the 81-step scan needs no transposes.
- Layer 1 (129->1000): fp32r (12-bit) hi/lo cross terms wh@xh + wh@xl +
  wl@xh + a K=5 combo matmul (last feature + 24-bit bias rows), 1 PE
  cycle/column each. The reset subtraction rides the psum group as a -I
  diag matmul so no elementwise pass needs the previous spikes.
- Layer 2 (1000->1000), the dominant cost: 4-term fp8(e4m3) cascade in
  DoubleRow pairs at 0.5 cycles/column and 252 contraction rows per pass
  (verified bit-for-bit on HW; effective weight error ~6e-8, measured
  zero output spike flips). Group 0 lhsT pairs (t0, t1*2^4) multiply rhs
  pairs (s1, s1*2^-4) packed by two ACT converts per tile; group 1 pairs
  (t2*2^12, t3*2^16) reuse the same rhs into a second psum, merged with
  *2^-12 in the membrane update.
- Layers 3/4: single 12-bit fp32r terms (measured flip-free) with 24-bit
  hi/lo bias rows; layer-2 spikes arrive as {0,2} so w3 is halved.
- LIF membrane updates are beta-fused pre-updates: one stt per tile
  m = beta*m + psum (+ the psum's embedded resets/biases), then the spike
  threshold; layer-2 subtracts -0.5*u_old in a second stt. Elementwise
  work is spread so DVE keeps the psum ops, Pool the thresholds/subs, and
  ACT the fp8 packs; per-tile chains pipeline across engines.
- Software pipelining: layer-1 psums + LIF + packs for step t+1 run inside
  iteration t; layer 4 of step t runs inside iteration t+1.
"""

import os
import sys

import numpy as np

for _p in ("/opt/trn_rl_repo", "/root/.axon_site/_ro/trn_rl_repo"):
    if os.path.isdir(_p) and _p not in sys.path:
        sys.path.insert(0, _p)

import concourse.bacc as bacc
import concourse.bass as bass
import concourse.mybir as mybir
import concourse.tile as tile
from concourse.bass_utils import run_bass_kernel_spmd
from concourse.tile_rust import add_dep_helper

# Problem constants (hardcoded; kernel.py must be self-contained).
T = 81          # time steps
F = 129         # input features per step
H = 1000        # hidden units (layers 1, 2)
HT = 125        # hidden tile rows  (H = 8 * 125)
NH = 8          # number of hidden tiles
H3 = 20         # layer-3 units
H4 = 10         # output units
BATCH = 4096
NCORES = 8
B = BATCH // NCORES   # 512 batch rows per core
BETA = 0.95
THRESH = 1.0
XR = 2 * 128 + 5      # x_aug rows: xh[0:128], xl[0:128], 5 combo rows

F32 = mybir.dt.float32
F32R = mybir.dt.float32r
F8 = mybir.dt.float8e4
AOP = mybir.AluOpType
DR = mybir.MatmulPerfMode.DoubleRow
MT = 128              # layer-2 output tile rows, padded to the 128 the
                      # dual-fp8 ldweights ISA check requires (H = 8*125)
HP = NH * MT          # padded layer-2 output columns in the fp8 lhsT


def build_bass():
    # Bacc (not raw Bass): its compile() runs generate_event_semaphores /
    # move_matmul_waits_to_ldweights, required because TRN2 Matmult
    # instructions can carry at most one sync wait.
    nc = bacc.Bacc(trn_type="TRN2", target_bir_lowering=False)

    x_d = nc.dram_tensor("x_aug", [T, XR, B], F32R, kind="ExternalInput")
    w1h_d = nc.dram_tensor("w1h", [128, H], F32R, kind="ExternalInput")
    w1l_d = nc.dram_tensor("w1l", [128, H], F32R, kind="ExternalInput")
    w1c_d = nc.dram_tensor("w1c", [5, H], F32R, kind="ExternalInput")
    # -identity: folds the layer-1 reset subtraction into the psum group,
    # keeping the Pool engine out of the per-step LIF critical chain
    l1d_d = nc.dram_tensor("l1d", [HT, HT], F32R, kind="ExternalInput")
    # layer-2 weights as a 4-term fp8(e4m3) cascade in DoubleRow pairs:
    # group 0 lhsT pairs (t0, t1*2^4) with rhs pairs (s, s*2^-4); group 1
    # pairs (t2*2^12, t3*2^16) share the same rhs, psum merged with *2^-12.
    # Effective weight error ~6e-8 (measured zero output spike flips), at
    # 0.5 PE cycles/column and 252 contraction rows per pass.
    w2p_d = nc.dram_tensor("w2p", [2, NH, HT + 2, 2, HP], F8,
                           kind="ExternalInput")
    # layer-3 weights: single 12-bit fp32r term (measured: zero output spike
    # flips end-to-end), 8 matmuls into one [20, B] psum, rhs is s2 directly.
    # Bias stays 24-bit via hi+lo rows 125/126 of the last k-tile (the s2
    # tile carries two ones-rows there).
    w3r_d = nc.dram_tensor("w3r", [NH, HT + 2, H3], F32R, kind="ExternalInput")
    # layer-4 lhsT: rows 0..19 w4 (12-bit, also flip-free), rows 20/21 bias
    # hi/lo riding ones-rows of the s3 tile
    w4c_d = nc.dram_tensor("w4c", [22, H4], F32R, kind="ExternalInput")
    outs_d = nc.dram_tensor("out_s", [T, H4, B], F32, kind="ExternalOutput")
    outm_d = nc.dram_tensor("out_m", [T, H4, B], F32, kind="ExternalOutput")

    with tile.TileContext(nc) as tc:
        with (
            tc.tile_pool(name="pers", bufs=1) as pers,
            tc.tile_pool(name="xpool", bufs=3) as xpool,
            tc.tile_pool(name="ps1", bufs=2, space="PSUM") as ps1,
            tc.tile_pool(name="ps2", bufs=2, space="PSUM") as ps2,
            tc.tile_pool(name="ps3", bufs=1, space="PSUM") as ps3,
            tc.tile_pool(name="ps4", bufs=1, space="PSUM") as ps4,
        ):
            # ---- persistent SBUF tensors ----
            w1h = pers.tile([128, H], F32R, tag="w1h")
            w1l = pers.tile([128, H], F32R, tag="w1l")
            w1c = pers.tile([5, H], F32R, tag="w1c")
            l1d = pers.tile([HT, HT], F32R, tag="l1d")
            w2p = pers.tile([HT + 2, 2, NH, 2, HP], F8, tag="w2p")
            s1pa = pers.tile([HT + 2, 2, NH * B], F8, tag="s1pa")
            s1pb = pers.tile([HT + 2, 2, NH * B], F8, tag="s1pb")
            s1pp = (s1pa, s1pb)   # indexed by step parity
            w3r = pers.tile([HT + 2, NH * H3], F32R, tag="w3r")  # [127, 160]
            w4c = pers.tile([22, H4], F32R, tag="w4c")
            m1 = pers.tile([HT, NH * B], F32, tag="m1")          # [125, 4096]
            m2 = pers.tile([HT, NH * B], F32, tag="m2")
            m3 = pers.tile([H3, B], F32, tag="m3")               # [20, 512]
            m4 = pers.tile([H4, B], F32, tag="m4")               # [10, 512]
            s1 = pers.tile([HT + 1, NH * B], F32R, tag="s1")     # [126, 4096]
            s2 = pers.tile([HT + 2, NH * B], F32R, tag="s2")     # [127, 4096]
            s3 = pers.tile([22, B], F32R, tag="s3")
            s4 = pers.tile([H4, B], F32, tag="s4")

            # fp32 views of the fp32r spike tiles for elementwise consumers
            s1f = s1[:].bitcast(F32)
            s2f = s2[:].bitcast(F32)
            s3f = s3[:].bitcast(F32)

            # ---- weight loads (layer-1 weights + x(0) first: they gate
            # step 0; the bulk w2/w3 transfers follow) ----
            def load_x(t):
                xh = xpool.tile([128, B], F32R, tag="xh", name="xh")
                xl = xpool.tile([128, B], F32R, tag="xl", name="xl")
                xc = xpool.tile([5, B], F32R, tag="xc", name="xc")
                nc.sync.dma_start(xh[:], x_d[t, 0:128, :])
                nc.sync.dma_start(xl[:], x_d[t, 128:256, :])
                nc.sync.dma_start(xc[:], x_d[t, 256:261, :])
                return xh, xl, xc

            w1dmas = []
            for sb, dr in [(w1h, w1h_d), (w1l, w1l_d), (w1c, w1c_d),
                           (l1d, l1d_d)]:
                w1dmas.append(nc.sync.dma_start(sb[:], dr[:]))
            x0 = load_x(0)
            wdmas = [nc.sync.dma_start(w4c[:], w4c_d[:])]
            for k in range(NH):
                for g in range(2):
                    wdmas.append(
                        nc.sync.dma_start(w2p[:, g, k, :, :], w2p_d[g, k]))
                wdmas.append(nc.sync.dma_start(
                    w3r[:, k * H3:(k + 1) * H3], w3r_d[k]))

            # Matmult instructions can carry at most ONE sync wait in the
            # TRN2 ISA (fp32/fp32r fuse the weight load into the matmul), so
            # have PE nops absorb the weight-DMA waits before any matmul.
            # Layer-1 absorbers go before the prologue; the rest only need to
            # precede the main loop's first layer-2/3/4 matmuls.
            def absorb(dmas):
                nops = []
                for d in dmas:
                    nop = nc.tensor.nop(nofuse=True)
                    add_dep_helper(nop.ins, d.ins, sync=True,
                                   reason="absorb weight-DMA wait on PE")
                    nops.append(nop)
                return nops

            absorbers = absorb(w1dmas)

            # ---- state init ----
            nc.vector.memset(m1[:], 0.0)
            nc.vector.memset(m2[:], 0.0)
            nc.gpsimd.memset(m3[:], 0.0)
            nc.gpsimd.memset(m4[:], 0.0)
            nc.gpsimd.memset(s4[:], 0.0)
            # ones rows feeding the bias fold (k-tile 7 / layer-4 rhs).
            # Engine ops need partition bases in {0,32,64,96}, so memset a
            # wider aligned region; all rows except the last are overwritten
            # by the per-step spike writes before any matmul reads them.
            nc.vector.memset(s1f[96:HT + 1, (NH - 1) * B:], 1.0)
            nc.vector.memset(s2f[96:HT + 2, (NH - 1) * B:], 1.0)
            # pre-update structure reads the previous step's spikes before
            # computing new ones, so spike rows need a zero init AFTER the
            # ones-row memsets above (row 125+ keeps the bias ones)
            nc.vector.memset(s1f[0:HT, :], 0.0)
            nc.vector.memset(s2f[0:HT, :], 0.0)
            nc.gpsimd.memset(s3f[:, :], 1.0)   # rows 20/21 stay as ones rows
            nc.gpsimd.memset(s3f[0:H3, :], 0.0)
            # fp8 rhs pack: row 125 holds the (1, 2^-4) constant pair that
            # multiplies the bias rows of the last k-tile; the ACT pack
            # copies overwrite rows 0..124 every step
            for s1p in s1pp:
                nc.vector.memset(s1p[96:HT + 2, 0, :], 1.0)
                nc.vector.memset(s1p[96:HT + 2, 1, :], 0.0625)

            def l1_block(xh, xl, xc, s1p):
                """Layer-1 psums + LIF + fp8 packs for one step, per tile.

                Per-tile chain: psum (incl -I*s1_old reset) -> m1 = b*m1+p
                (DVE stt) -> spike (Pool is_gt) -> fp8 packs (ACT), so each
                engine touches a tile once and tiles pipeline."""
                first_mm = None
                for h in range(NH):
                    p1 = ps1.tile([HT, B], F32, tag="p1")
                    c0 = h * HT
                    cols = slice(h * B, (h + 1) * B)
                    mm = nc.tensor.matmul(p1[:], l1d[:], s1[0:HT, cols],
                                          start=True, stop=False)
                    if first_mm is None:
                        first_mm = mm
                    nc.tensor.matmul(p1[:], w1h[:, c0:c0 + HT], xh[:],
                                     start=False, stop=False)
                    nc.tensor.matmul(p1[:], w1h[:, c0:c0 + HT], xl[:],
                                     start=False, stop=False)
                    nc.tensor.matmul(p1[:], w1l[:, c0:c0 + HT], xh[:],
                                     start=False, stop=False)
                    nc.tensor.matmul(p1[:], w1c[:, c0:c0 + HT], xc[:],
                                     start=False, stop=True)
                    nc.vector.scalar_tensor_tensor(m1[:, cols], m1[:, cols],
                                                   BETA, p1[:],
                                                   AOP.mult, AOP.add)
                    nc.gpsimd.tensor_scalar(s1[0:HT, cols], m1[:, cols],
                                            THRESH, None, AOP.is_gt)
                    nc.scalar.copy(s1p[0:HT, 0, cols], s1f[0:HT, cols])
                    nc.scalar.activation(s1p[0:HT, 1, cols], s1f[0:HT, cols],
                                         mybir.ActivationFunctionType.Copy,
                                         scale=0.0625)
                return first_mm

            # ---- prologue: step 0 layer-1 ----
            first_mm = l1_block(*x0, s1pp[0])
            for nop in absorbers:
                add_dep_helper(first_mm.ins, nop.ins, sync=False,
                               reason="keep absorbers before first matmul")

            # absorb the remaining weight DMAs before the main loop's
            # layer-2/3/4 matmuls
            late_absorbers = absorb(wdmas)

            def l4_block(t):
                """Layer 4 for step t + LIF + output DMAs."""
                p4 = ps4.tile([H4, B], F32, tag="p4")
                nc.tensor.matmul(p4[:], w4c[:], s3[:], start=True, stop=True)
                nc.vector.scalar_tensor_tensor(m4[:], m4[:], BETA, p4[:],
                                               AOP.mult, AOP.add)
                nc.gpsimd.tensor_tensor(m4[:], m4[:], s4[:], AOP.subtract)
                nc.sync.dma_start(outm_d[t], m4[:])
                nc.gpsimd.tensor_scalar(s4[:], m4[:], THRESH, None, AOP.is_gt)
                nc.sync.dma_start(outs_d[t], s4[:])

            # ---- main loop over steps ----
            for i in range(T):
                if i < T - 1:
                    xh, xl, xc = load_x(i + 1)

                # layer 2 of step i: fp8 DoubleRow, two psum scale-groups;
                # each pass consumes 252 contraction rows at 256 PE cycles
                s1p = s1pp[i % 2]
                for h in range(NH):
                    c0 = h * MT
                    pA = ps2.tile([MT, B], F32, tag="pA")
                    pB = ps2.tile([MT, B], F32, tag="pB")
                    for g, pX in ((0, pA), (1, pB)):
                        for k in range(NH):
                            mm2 = nc.tensor.matmul(
                                pX[:],
                                w2p[0:HT + 2, g, k, :, c0:c0 + MT],
                                s1p[0:HT + 2, :, k * B:(k + 1) * B],
                                start=(k == 0),
                                stop=(k == NH - 1),
                                perf_mode=DR)
                            if i == 0 and h == 0 and k == 0 and g == 0:
                                for nop in late_absorbers:
                                    add_dep_helper(
                                        mm2.ins, nop.ins, sync=False,
                                        reason="absorbers before first L2 mm")
                    cols = slice(h * B, (h + 1) * B)
                    # m2 = beta*m2 + psumA + psumB*2^-12 - u_old/2 with
                    # u = 2*(m2>1); layer-3 weights are halved host-side.
                    nc.vector.scalar_tensor_tensor(m2[:, cols], m2[:, cols],
                                                   BETA, pA[0:HT, :],
                                                   AOP.mult, AOP.add)
                    nc.vector.scalar_tensor_tensor(m2[:, cols], pB[0:HT, :],
                                                   2.0 ** -12, m2[:, cols],
                                                   AOP.mult, AOP.add)
                    nc.vector.scalar_tensor_tensor(m2[:, cols],
                                                   s2f[0:HT, cols], -0.5,
                                                   m2[:, cols],
                                                   AOP.mult, AOP.add)
                    # spikes as {0, 2}: exact is_gt (ACT Sign rounds near
                    # the threshold), then *2 in the same instruction; the
                    # halved layer-3 weights and the -0.5 subtract coeff
                    # make u/2 the effective 0/1 spike
                    nc.gpsimd.tensor_scalar(s2[0:HT, cols], m2[:, cols],
                                            THRESH, 2.0, AOP.is_gt, AOP.mult)

                # layer 4 of step i-1 (deferred so spk3 is long ready)
                if i > 0:
                    l4_block(i - 1)

                # layer-1 psums + LIF + packs for step i+1 (into the
                # other pack buffer, so packs overlap this step's layer 2)
                if i < T - 1:
                    l1_block(xh, xl, xc, s1pp[(i + 1) % 2])

                # layer 3 of step i: single 12-bit fp32r term (weights
                # pre-halved for the +-1 spike convention), 8 matmuls into
                # one [20, B] psum
                p3 = ps3.tile([H3, B], F32, tag="p3")
                for k in range(NH):
                    kk = HT + 2 if k == NH - 1 else HT
                    nc.tensor.matmul(
                        p3[:],
                        w3r[0:kk, k * H3:(k + 1) * H3],
                        s2[0:kk, k * B:(k + 1) * B],
                        start=(k == 0),
                        stop=(k == NH - 1))
                nc.vector.scalar_tensor_tensor(m3[:], m3[:], BETA, p3[:],
                                               AOP.mult, AOP.add)
                nc.gpsimd.tensor_tensor(m3[:], m3[:], s3f[0:H3, :],
                                        AOP.subtract)
                nc.gpsimd.tensor_scalar(s3[0:H3, :], m3[:], THRESH, None,
                                        AOP.is_gt)

            # ---- epilogue ----
            l4_block(T - 1)

    nc.compile()
    return nc


_CACHE = {}


def _get_nc():
    if "nc" not in _CACHE:
        _CACHE["nc"] = build_bass()
    return _CACHE["nc"]


def _rne12(a):
    """Round fp32 to 12 significand bits (the fp32r grid), RNE —
    bit-identical to the device's fp32r rounding."""
    drop = np.uint64(12)
    u = np.ascontiguousarray(a, np.float32).view(np.uint32).astype(np.uint64)
    half = np.uint64(1 << 11)
    lsb = (u >> drop) & np.uint64(1)
    u2 = ((u + half - np.uint64(1) + lsb) >> drop << drop)
    return u2.astype(np.uint32).view(np.float32).reshape(a.shape)


def _hilo(a):
    hi = _rne12(a)
    lo = _rne12(np.asarray(a, np.float32) - hi)
    return hi, lo


def _prep_inputs(x, w1, b1, w2, b2, w3, b3, w4, b4):
    x = np.ascontiguousarray(x, np.float32)
    # xs[t, f, b_global]; step t of the reference reads x[:, f*T + t]
    xt = np.ascontiguousarray(
        np.transpose(x.reshape(BATCH, F, T), (2, 1, 0)))   # [T, F, BATCH]
    xth, xtl = _hilo(xt)

    w1T = np.ascontiguousarray(w1.T.astype(np.float32))    # [129, 1000]
    w1h, w1l = _hilo(w1T[:128])
    whL, wlL = _hilo(w1T[128])
    b1h, b1l = _hilo(b1.astype(np.float32))
    w1c = np.stack([whL, whL, wlL, b1h, b1l])              # [5, 1000]

    # layer-2: 4-term e4m3 cascade of [w2.T; b2] -> DoubleRow-paired lhsT.
    # Terms t0..t3 at effective scales 1, 2^-4, 2^-12, 2^-16; groups
    # (t0, t1*2^4) and (t2*2^12, t3*2^16) pair with rhs (s, s*2^-4).
    import ml_dtypes
    e4 = ml_dtypes.float8_e4m3

    def q8(a):
        return a.astype(e4).astype(np.float32)

    wb2 = np.concatenate([np.ascontiguousarray(w2.T.astype(np.float32)),
                          b2.astype(np.float32)[None, :]])   # [1001, 1000]
    t0 = q8(wb2)
    r = wb2 - t0
    t1 = q8(r * 2.0 ** 4)
    r = r - t1 * 2.0 ** -4
    t2 = q8(r * 2.0 ** 12)
    r = r - t2 * 2.0 ** -12
    t3 = q8(r * 2.0 ** 16)
    w2p = np.zeros((2, NH, HT + 2, 2, NH * 128), e4)
    terms = ((t0, t1), (t2, t3))
    for g in range(2):
        for k in range(NH):
            for i2 in range(2):
                src = terms[g][i2][k * HT:(k + 1) * HT]      # [125, 1000]
                for h in range(NH):
                    w2p[g, k, :HT, i2, h * 128:h * 128 + HT] = (
                        src[:, h * HT:(h + 1) * HT])
        for i2 in range(2):
            brow = terms[g][i2][H]                           # bias row
            for h in range(NH):
                w2p[g, NH - 1, HT, i2, h * 128:h * 128 + HT] = (
                    brow[h * HT:(h + 1) * HT])

    # layer-3: single 12-bit term; layer-2 spikes arrive as {0,2}, so the
    # weights are halved (exact) and the bias stays plain 24-bit hi/lo
    w3q = _rne12(np.ascontiguousarray(w3.T.astype(np.float32)) * 0.5)
    b3h, b3l = _hilo(b3.astype(np.float32))
    w3r = np.zeros((NH, HT + 2, H3), np.float32)
    for k in range(NH):
        w3r[k, :HT] = w3q[k * HT:(k + 1) * HT]
    w3r[NH - 1, HT] = b3h
    w3r[NH - 1, HT + 1] = b3l

    l1d = np.ascontiguousarray(-np.eye(HT, dtype=np.float32))

    b4h, b4l = _hilo(b4.astype(np.float32))
    w4c = np.zeros((22, H4), np.float32)
    w4c[0:20] = _rne12(w4.T.astype(np.float32))
    w4c[20] = b4h
    w4c[21] = b4l

    in_maps = []
    for c in range(NCORES):
        xc = np.empty((T, XR, B), np.float32)
        xc[:, 0:128, :] = xth[:, 0:128, c * B:(c + 1) * B]
        xc[:, 128:256, :] = xtl[:, 0:128, c * B:(c + 1) * B]
        xc[:, 256, :] = xth[:, 128, c * B:(c + 1) * B]
        xc[:, 257, :] = xtl[:, 128, c * B:(c + 1) * B]
        xc[:, 258, :] = xth[:, 128, c * B:(c + 1) * B]
        xc[:, 259, :] = 1.0
        xc[:, 260, :] = 1.0
        in_maps.append({
            "x_aug": xc, "w1h": w1h, "w1l": w1l, "w1c": w1c, "l1d": l1d,
            "w2p": w2p, "w3r": w3r, "w4c": w4c,
        })
    return in_maps


def _gather(results):
    spk = np.concatenate(
        [np.transpose(r["out_s"], (0, 2, 1)) for r in results], axis=1)
    mem = np.concatenate(
        [np.transpose(r["out_m"], (0, 2, 1)) for r in results], axis=1)
    return spk, mem


def kernel(x, w1, b1, w2, b2, w3, b3, w4, b4, _trace=False, _trace_kwargs=None):
    # accept numpy or jax arrays, any float dtype
    x, w1, b1, w2, b2, w3, b3, w4, b4 = (
        np.asarray(a, dtype=np.float32)
        for a in (x, w1, b1, w2, b2, w3, b3, w4, b4))
    nc = _get_nc()
    in_maps = _prep_inputs(x, w1, b1, w2, b2, w3, b3, w4, b4)
    res = run_bass_kernel_spmd(
        nc, in_maps, core_ids=list(range(NCORES)),
        trace=_trace, **(_trace_kwargs or {}))
    out = _gather(res.results)
    if _trace:
        return out, res
    return out



# revision 52
# speedup vs baseline: 1.0036x; 1.0036x over previous
"""Trainium2 Bass kernel for the AudNet 4-layer LIF spiking network.

Reference computation (per time step t of 81, batch 4096):
    s1, m1 = lif(x_t @ w1.T + b1, m1)     # 129 -> 1000
    s2, m2 = lif(s1 @ w2.T + b2, m2)      # 1000 -> 1000
    s3, m3 = lif(s2 @ w3.T + b3, m3)      # 1000 -> 20
    s4, m4 = lif(s3 @ w4.T + b4, m4)      # 20 -> 10
with lif: reset = (m > 1); m' = 0.95*m + cur - reset; spk = (m' > 1)
Outputs: (s4, m4) per step -> each [81, 4096, 10].

Strategy:
- Data parallel over 8 NeuronCores: 512 batch rows per core; weights
  replicated; no cross-device traffic.
- Hidden-on-partition, batch-on-free layout: weights are the stationary
  lhsT, spikes/x the moving rhs; the 81-step scan needs no transposes.
- Layer 1 (129->1000): fp32r (12-bit) hi/lo cross terms wh@xh + wh@xl +
  wl@xh + a K=5 combo matmul (last feature + 24-bit bias rows), 1 PE
  cycle/column each. The reset subtraction rides the psum group as a -I
  diag matmul so no elementwise pass needs the previous spikes.
- Layer 2 (1000->1000), the dominant cost: 4-term fp8(e4m3) cascade in
  DoubleRow pairs at 0.5 cycles/column and 252 contraction rows per pass
  (verified bit-for-bit on HW; effective weight error ~6e-8, measured
  zero output spike flips). Group 0 lhsT pairs (t0, t1*2^4) multiply rhs
  pairs (s1, s1*2^-4) packed by two ACT converts per tile; group 1 pairs
  (t2*2^12, t3*2^16) reuse the same rhs into a second psum, merged with
  *2^-12 in the membrane update.
- Layers 3/4: single 12-bit fp32r terms (measured flip-free) with 24-bit
  hi/lo bias rows; layer-2 spikes arrive as {0,2} so w3 is halved.
- LIF membrane updates are beta-fused pre-updates: one stt per tile
  m = beta*m + psum (+ the psum's embedded resets/biases), then the spike
  threshold; layer-2 subtracts -0.5*u_old in a second stt. Elementwise
  work is spread so DVE keeps the psum ops, Pool the thresholds/subs, and
  ACT the fp8 packs; per-tile chains pipeline across engines.
- Software pipelining: layer-1 psums + LIF + packs for step t+1 run inside
  iteration t; layer 4 of step t runs inside iteration t+1.
"""

import os
import sys

import numpy as np

for _p in ("/opt/trn_rl_repo", "/root/.axon_site/_ro/trn_rl_repo"):
    if os.path.isdir(_p) and _p not in sys.path:
        sys.path.insert(0, _p)

import concourse.bacc as bacc
import concourse.bass as bass
import concourse.mybir as mybir
import concourse.tile as tile
from concourse.bass_utils import run_bass_kernel_spmd
from concourse.tile_rust import add_dep_helper

# Problem constants (hardcoded; kernel.py must be self-contained).
T = 81          # time steps
F = 129         # input features per step
H = 1000        # hidden units (layers 1, 2)
HT = 125        # hidden tile rows  (H = 8 * 125)
NH = 8          # number of hidden tiles
H3 = 20         # layer-3 units
H4 = 10         # output units
BATCH = 4096
NCORES = 8
B = BATCH // NCORES   # 512 batch rows per core
BETA = 0.95
THRESH = 1.0
XR = 2 * 128 + 5      # x_aug rows: xh[0:128], xl[0:128], 5 combo rows

F32 = mybir.dt.float32
F32R = mybir.dt.float32r
F8 = mybir.dt.float8e4
AOP = mybir.AluOpType
DR = mybir.MatmulPerfMode.DoubleRow
MT = 128              # layer-2 output tile rows, padded to the 128 the
                      # dual-fp8 ldweights ISA check requires (H = 8*125)
HP = NH * MT          # padded layer-2 output columns in the fp8 lhsT


def build_bass():
    # Bacc (not raw Bass): its compile() runs generate_event_semaphores /
    # move_matmul_waits_to_ldweights, required because TRN2 Matmult
    # instructions can carry at most one sync wait.
    nc = bacc.Bacc(trn_type="TRN2", target_bir_lowering=False)

    x_d = nc.dram_tensor("x_aug", [T, XR, B], F32R, kind="ExternalInput")
    w1h_d = nc.dram_tensor("w1h", [128, H], F32R, kind="ExternalInput")
    w1l_d = nc.dram_tensor("w1l", [128, H], F32R, kind="ExternalInput")
    w1c_d = nc.dram_tensor("w1c", [5, H], F32R, kind="ExternalInput")
    # -identity: folds the layer-1 reset subtraction into the psum group,
    # keeping the Pool engine out of the per-step LIF critical chain
    l1d_d = nc.dram_tensor("l1d", [HT, HT], F32R, kind="ExternalInput")
    # layer-2 weights as a 4-term fp8(e4m3) cascade in DoubleRow pairs:
    # group 0 lhsT pairs (t0, t1*2^4) with rhs pairs (s, s*2^-4); group 1
    # pairs (t2*2^12, t3*2^16) share the same rhs, psum merged with *2^-12.
    # Effective weight error ~6e-8 (measured zero output spike flips), at
    # 0.5 PE cycles/column and 252 contraction rows per pass.
    w2p_d = nc.dram_tensor("w2p", [2, NH, HT + 2, 2, HP], F8,
                           kind="ExternalInput")
    # layer-3 weights: single 12-bit fp32r term (measured: zero output spike
    # flips end-to-end), 8 matmuls into one [20, B] psum, rhs is s2 directly.
    # Bias stays 24-bit via hi+lo rows 125/126 of the last k-tile (the s2
    # tile carries two ones-rows there).
    w3r_d = nc.dram_tensor("w3r", [NH, HT + 2, H3], F32R, kind="ExternalInput")
    # layer-4 lhsT: rows 0..19 w4 (12-bit, also flip-free), rows 20/21 bias
    # hi/lo riding ones-rows of the s3 tile
    w4c_d = nc.dram_tensor("w4c", [22, H4], F32R, kind="ExternalInput")
    outs_d = nc.dram_tensor("out_s", [T, H4, B], F32, kind="ExternalOutput")
    outm_d = nc.dram_tensor("out_m", [T, H4, B], F32, kind="ExternalOutput")

    with tile.TileContext(nc) as tc:
        with (
            tc.tile_pool(name="pers", bufs=1) as pers,
            tc.tile_pool(name="xpool", bufs=3) as xpool,
            tc.tile_pool(name="ps1", bufs=2, space="PSUM") as ps1,
            tc.tile_pool(name="ps2", bufs=2, space="PSUM") as ps2,
            tc.tile_pool(name="ps3", bufs=1, space="PSUM") as ps3,
            tc.tile_pool(name="ps4", bufs=1, space="PSUM") as ps4,
        ):
            # ---- persistent SBUF tensors ----
            w1h = pers.tile([128, H], F32R, tag="w1h")
            w1l = pers.tile([128, H], F32R, tag="w1l")
            w1c = pers.tile([5, H], F32R, tag="w1c")
            l1d = pers.tile([HT, HT], F32R, tag="l1d")
            w2p = pers.tile([HT + 2, 2, NH, 2, HP], F8, tag="w2p")
            s1p = pers.tile([HT + 2, 2, NH * B], F8, tag="s1p")
            w3r = pers.tile([HT + 2, NH * H3], F32R, tag="w3r")  # [127, 160]
            w4c = pers.tile([22, H4], F32R, tag="w4c")
            m1 = pers.tile([HT, NH * B], F32, tag="m1")          # [125, 4096]
            m2 = pers.tile([HT, NH * B], F32, tag="m2")
            m3 = pers.tile([H3, B], F32, tag="m3")               # [20, 512]
            m4 = pers.tile([H4, B], F32, tag="m4")               # [10, 512]
            s1 = pers.tile([HT + 1, NH * B], F32R, tag="s1")     # [126, 4096]
            s2 = pers.tile([HT + 2, NH * B], F32R, tag="s2")     # [127, 4096]
            s3 = pers.tile([22, B], F32R, tag="s3")
            s4 = pers.tile([H4, B], F32, tag="s4")

            # fp32 views of the fp32r spike tiles for elementwise consumers
            s1f = s1[:].bitcast(F32)
            s2f = s2[:].bitcast(F32)
            s3f = s3[:].bitcast(F32)

            # ---- weight loads (layer-1 weights + x(0) first: they gate
            # step 0; the bulk w2/w3 transfers follow) ----
            def load_x(t):
                xh = xpool.tile([128, B], F32R, tag="xh", name="xh")
                xl = xpool.tile([128, B], F32R, tag="xl", name="xl")
                xc = xpool.tile([5, B], F32R, tag="xc", name="xc")
                nc.sync.dma_start(xh[:], x_d[t, 0:128, :])
                nc.sync.dma_start(xl[:], x_d[t, 128:256, :])
                nc.sync.dma_start(xc[:], x_d[t, 256:261, :])
                return xh, xl, xc

            w1dmas = []
            for sb, dr in [(w1h, w1h_d), (w1l, w1l_d), (w1c, w1c_d),
                           (l1d, l1d_d)]:
                w1dmas.append(nc.sync.dma_start(sb[:], dr[:]))
            x0 = load_x(0)
            wdmas = [nc.sync.dma_start(w4c[:], w4c_d[:])]
            for k in range(NH):
                for g in range(2):
                    wdmas.append(
                        nc.sync.dma_start(w2p[:, g, k, :, :], w2p_d[g, k]))
                wdmas.append(nc.sync.dma_start(
                    w3r[:, k * H3:(k + 1) * H3], w3r_d[k]))

            # Matmult instructions can carry at most ONE sync wait in the
            # TRN2 ISA (fp32/fp32r fuse the weight load into the matmul), so
            # have PE nops absorb the weight-DMA waits before any matmul.
            # Layer-1 absorbers go before the prologue; the rest only need to
            # precede the main loop's first layer-2/3/4 matmuls.
            def absorb(dmas):
                nops = []
                for d in dmas:
                    nop = nc.tensor.nop(nofuse=True)
                    add_dep_helper(nop.ins, d.ins, sync=True,
                                   reason="absorb weight-DMA wait on PE")
                    nops.append(nop)
                return nops

            absorbers = absorb(w1dmas)

            # ---- state init ----
            nc.vector.memset(m1[:], 0.0)
            nc.vector.memset(m2[:], 0.0)
            nc.gpsimd.memset(m3[:], 0.0)
            nc.gpsimd.memset(m4[:], 0.0)
            nc.gpsimd.memset(s4[:], 0.0)
            # ones rows feeding the bias fold (k-tile 7 / layer-4 rhs).
            # Engine ops need partition bases in {0,32,64,96}, so memset a
            # wider aligned region; all rows except the last are overwritten
            # by the per-step spike writes before any matmul reads them.
            nc.vector.memset(s1f[96:HT + 1, (NH - 1) * B:], 1.0)
            nc.vector.memset(s2f[96:HT + 2, (NH - 1) * B:], 1.0)
            # pre-update structure reads the previous step's spikes before
            # computing new ones, so spike rows need a zero init AFTER the
            # ones-row memsets above (row 125+ keeps the bias ones)
            nc.vector.memset(s1f[0:HT, :], 0.0)
            nc.vector.memset(s2f[0:HT, :], 0.0)
            nc.gpsimd.memset(s3f[:, :], 1.0)   # rows 20/21 stay as ones rows
            nc.gpsimd.memset(s3f[0:H3, :], 0.0)
            # fp8 rhs pack: row 125 holds the (1, 2^-4) constant pair that
            # multiplies the bias rows of the last k-tile; the ACT pack
            # copies overwrite rows 0..124 every step
            nc.vector.memset(s1p[96:HT + 2, 0, :], 1.0)
            nc.vector.memset(s1p[96:HT + 2, 1, :], 0.0625)

            def l1_block(xh, xl, xc):
                """Layer-1 psums + LIF + fp8 packs for one step, per tile.

                Per-tile chain: psum (incl -I*s1_old reset) -> m1 = b*m1+p
                (DVE stt) -> spike (Pool is_gt) -> fp8 packs (ACT), so each
                engine touches a tile once and tiles pipeline."""
                first_mm = None
                for h in range(NH):
                    p1 = ps1.tile([HT, B], F32, tag="p1")
                    c0 = h * HT
                    cols = slice(h * B, (h + 1) * B)
                    mm = nc.tensor.matmul(p1[:], l1d[:], s1[0:HT, cols],
                                          start=True, stop=False)
                    if first_mm is None:
                        first_mm = mm
                    nc.tensor.matmul(p1[:], w1h[:, c0:c0 + HT], xh[:],
                                     start=False, stop=False)
                    nc.tensor.matmul(p1[:], w1h[:, c0:c0 + HT], xl[:],
                                     start=False, stop=False)
                    nc.tensor.matmul(p1[:], w1l[:, c0:c0 + HT], xh[:],
                                     start=False, stop=False)
                    nc.tensor.matmul(p1[:], w1c[:, c0:c0 + HT], xc[:],
                                     start=False, stop=True)
                    nc.vector.scalar_tensor_tensor(m1[:, cols], m1[:, cols],
                                                   BETA, p1[:],
                                                   AOP.mult, AOP.add)
                    nc.gpsimd.tensor_scalar(s1[0:HT, cols], m1[:, cols],
                                            THRESH, None, AOP.is_gt)
                    nc.scalar.copy(s1p[0:HT, 0, cols], s1f[0:HT, cols])
                    nc.scalar.activation(s1p[0:HT, 1, cols], s1f[0:HT, cols],
                                         mybir.ActivationFunctionType.Copy,
                                         scale=0.0625)
                return first_mm

            # ---- prologue: step 0 layer-1 ----
            first_mm = l1_block(*x0)
            for nop in absorbers:
                add_dep_helper(first_mm.ins, nop.ins, sync=False,
                               reason="keep absorbers before first matmul")

            # absorb the remaining weight DMAs before the main loop's
            # layer-2/3/4 matmuls
            late_absorbers = absorb(wdmas)

            def l4_block(t):
                """Layer 4 for step t + LIF + output DMAs."""
                p4 = ps4.tile([H4, B], F32, tag="p4")
                nc.tensor.matmul(p4[:], w4c[:], s3[:], start=True, stop=True)
                nc.vector.scalar_tensor_tensor(m4[:], m4[:], BETA, p4[:],
                                               AOP.mult, AOP.add)
                nc.gpsimd.tensor_tensor(m4[:], m4[:], s4[:], AOP.subtract)
                nc.sync.dma_start(outm_d[t], m4[:])
                nc.gpsimd.tensor_scalar(s4[:], m4[:], THRESH, None, AOP.is_gt)
                nc.sync.dma_start(outs_d[t], s4[:])

            # ---- main loop over steps ----
            for i in range(T):
                if i < T - 1:
                    xh, xl, xc = load_x(i + 1)

                # layer 2 of step i: fp8 DoubleRow, two psum scale-groups;
                # each pass consumes 252 contraction rows at 256 PE cycles
                for h in range(NH):
                    c0 = h * MT
                    pA = ps2.tile([MT, B], F32, tag="pA")
                    pB = ps2.tile([MT, B], F32, tag="pB")
                    for g, pX in ((0, pA), (1, pB)):
                        for k in range(NH):
                            mm2 = nc.tensor.matmul(
                                pX[:],
                                w2p[0:HT + 2, g, k, :, c0:c0 + MT],
                                s1p[0:HT + 2, :, k * B:(k + 1) * B],
                                start=(k == 0),
                                stop=(k == NH - 1),
                                perf_mode=DR)
                            if i == 0 and h == 0 and k == 0 and g == 0:
                                for nop in late_absorbers:
                                    add_dep_helper(
                                        mm2.ins, nop.ins, sync=False,
                                        reason="absorbers before first L2 mm")
                    cols = slice(h * B, (h + 1) * B)
                    # m2 = beta*m2 + psumA + psumB*2^-12 - u_old/2 with
                    # u = 2*(m2>1); layer-3 weights are halved host-side.
                    nc.vector.scalar_tensor_tensor(m2[:, cols], m2[:, cols],
                                                   BETA, pA[0:HT, :],
                                                   AOP.mult, AOP.add)
                    nc.vector.scalar_tensor_tensor(m2[:, cols], pB[0:HT, :],
                                                   2.0 ** -12, m2[:, cols],
                                                   AOP.mult, AOP.add)
                    nc.vector.scalar_tensor_tensor(m2[:, cols],
                                                   s2f[0:HT, cols], -0.5,
                                                   m2[:, cols],
                                                   AOP.mult, AOP.add)
                    # spikes as {0, 2}: exact is_gt (ACT Sign rounds near
                    # the threshold), then *2 in the same instruction; the
                    # halved layer-3 weights and the -0.5 subtract coeff
                    # make u/2 the effective 0/1 spike
                    nc.gpsimd.tensor_scalar(s2[0:HT, cols], m2[:, cols],
                                            THRESH, 2.0, AOP.is_gt, AOP.mult)

                # layer 4 of step i-1 (deferred so spk3 is long ready)
                if i > 0:
                    l4_block(i - 1)

                # layer-1 psums + LIF + packs for step i+1
                if i < T - 1:
                    l1_block(xh, xl, xc)

                # layer 3 of step i: single 12-bit fp32r term (weights
                # pre-halved for the +-1 spike convention), 8 matmuls into
                # one [20, B] psum
                p3 = ps3.tile([H3, B], F32, tag="p3")
                for k in range(NH):
                    kk = HT + 2 if k == NH - 1 else HT
                    nc.tensor.matmul(
                        p3[:],
                        w3r[0:kk, k * H3:(k + 1) * H3],
                        s2[0:kk, k * B:(k + 1) * B],
                        start=(k == 0),
                        stop=(k == NH - 1))
                nc.vector.scalar_tensor_tensor(m3[:], m3[:], BETA, p3[:],
                                               AOP.mult, AOP.add)
                nc.gpsimd.tensor_tensor(m3[:], m3[:], s3f[0:H3, :],
                                        AOP.subtract)
                nc.gpsimd.tensor_scalar(s3[0:H3, :], m3[:], THRESH, None,
                                        AOP.is_gt)

            # ---- epilogue ----
            l4_block(T - 1)

    nc.compile()
    return nc


_CACHE = {}


def _get_nc():
    if "nc" not in _CACHE:
        _CACHE["nc"] = build_bass()
    return _CACHE["nc"]


def _rne12(a):
    """Round fp32 to 12 significand bits (the fp32r grid), RNE —
    bit-identical to the device's fp32r rounding."""
    drop = np.uint64(12)
    u = np.ascontiguousarray(a, np.float32).view(np.uint32).astype(np.uint64)
    half = np.uint64(1 << 11)
    lsb = (u >> drop) & np.uint64(1)
    u2 = ((u + half - np.uint64(1) + lsb) >> drop << drop)
    return u2.astype(np.uint32).view(np.float32).reshape(a.shape)


def _hilo(a):
    hi = _rne12(a)
    lo = _rne12(np.asarray(a, np.float32) - hi)
    return hi, lo


def _prep_inputs(x, w1, b1, w2, b2, w3, b3, w4, b4):
    x = np.ascontiguousarray(x, np.float32)
    # xs[t, f, b_global]; step t of the reference reads x[:, f*T + t]
    xt = np.ascontiguousarray(
        np.transpose(x.reshape(BATCH, F, T), (2, 1, 0)))   # [T, F, BATCH]
    xth, xtl = _hilo(xt)

    w1T = np.ascontiguousarray(w1.T.astype(np.float32))    # [129, 1000]
    w1h, w1l = _hilo(w1T[:128])
    whL, wlL = _hilo(w1T[128])
    b1h, b1l = _hilo(b1.astype(np.float32))
    w1c = np.stack([whL, whL, wlL, b1h, b1l])              # [5, 1000]

    # layer-2: 4-term e4m3 cascade of [w2.T; b2] -> DoubleRow-paired lhsT.
    # Terms t0..t3 at effective scales 1, 2^-4, 2^-12, 2^-16; groups
    # (t0, t1*2^4) and (t2*2^12, t3*2^16) pair with rhs (s, s*2^-4).
    import ml_dtypes
    e4 = ml_dtypes.float8_e4m3

    def q8(a):
        return a.astype(e4).astype(np.float32)

    wb2 = np.concatenate([np.ascontiguousarray(w2.T.astype(np.float32)),
                          b2.astype(np.float32)[None, :]])   # [1001, 1000]
    t0 = q8(wb2)
    r = wb2 - t0
    t1 = q8(r * 2.0 ** 4)
    r = r - t1 * 2.0 ** -4
    t2 = q8(r * 2.0 ** 12)
    r = r - t2 * 2.0 ** -12
    t3 = q8(r * 2.0 ** 16)
    w2p = np.zeros((2, NH, HT + 2, 2, NH * 128), e4)
    terms = ((t0, t1), (t2, t3))
    for g in range(2):
        for k in range(NH):
            for i2 in range(2):
                src = terms[g][i2][k * HT:(k + 1) * HT]      # [125, 1000]
                for h in range(NH):
                    w2p[g, k, :HT, i2, h * 128:h * 128 + HT] = (
                        src[:, h * HT:(h + 1) * HT])
        for i2 in range(2):
            brow = terms[g][i2][H]                           # bias row
            for h in range(NH):
                w2p[g, NH - 1, HT, i2, h * 128:h * 128 + HT] = (
                    brow[h * HT:(h + 1) * HT])

    # layer-3: single 12-bit term; layer-2 spikes arrive as {0,2}, so the
    # weights are halved (exact) and the bias stays plain 24-bit hi/lo
    w3q = _rne12(np.ascontiguousarray(w3.T.astype(np.float32)) * 0.5)
    b3h, b3l = _hilo(b3.astype(np.float32))
    w3r = np.zeros((NH, HT + 2, H3), np.float32)
    for k in range(NH):
        w3r[k, :HT] = w3q[k * HT:(k + 1) * HT]
    w3r[NH - 1, HT] = b3h
    w3r[NH - 1, HT + 1] = b3l

    l1d = np.ascontiguousarray(-np.eye(HT, dtype=np.float32))

    b4h, b4l = _hilo(b4.astype(np.float32))
    w4c = np.zeros((22, H4), np.float32)
    w4c[0:20] = _rne12(w4.T.astype(np.float32))
    w4c[20] = b4h
    w4c[21] = b4l

    in_maps = []
    for c in range(NCORES):
        xc = np.empty((T, XR, B), np.float32)
        xc[:, 0:128, :] = xth[:, 0:128, c * B:(c + 1) * B]
        xc[:, 128:256, :] = xtl[:, 0:128, c * B:(c + 1) * B]
        xc[:, 256, :] = xth[:, 128, c * B:(c + 1) * B]
        xc[:, 257, :] = xtl[:, 128, c * B:(c + 1) * B]
        xc[:, 258, :] = xth[:, 128, c * B:(c + 1) * B]
        xc[:, 259, :] = 1.0
        xc[:, 260, :] = 1.0
        in_maps.append({
            "x_aug": xc, "w1h": w1h, "w1l": w1l, "w1c": w1c, "l1d": l1d,
            "w2p": w2p, "w3r": w3r, "w4c": w4c,
        })
    return in_maps


def _gather(results):
    spk = np.concatenate(
        [np.transpose(r["out_s"], (0, 2, 1)) for r in results], axis=1)
    mem = np.concatenate(
        [np.transpose(r["out_m"], (0, 2, 1)) for r in results], axis=1)
    return spk, mem


def kernel(x, w1, b1, w2, b2, w3, b3, w4, b4, _trace=False, _trace_kwargs=None):
    # accept numpy or jax arrays, any float dtype
    x, w1, b1, w2, b2, w3, b3, w4, b4 = (
        np.asarray(a, dtype=np.float32)
        for a in (x, w1, b1, w2, b2, w3, b3, w4, b4))
    nc = _get_nc()
    in_maps = _prep_inputs(x, w1, b1, w2, b2, w3, b3, w4, b4)
    res = run_bass_kernel_spmd(
        nc, in_maps, core_ids=list(range(NCORES)),
        trace=_trace, **(_trace_kwargs or {}))
    out = _gather(res.results)
    if _trace:
        return out, res
    return out



# revision 55
# speedup vs baseline: 1.0698x; 1.0659x over previous
"""Trainium2 Bass kernel for the AudNet 4-layer LIF spiking network.

Reference computation (per time step t of 81, batch 4096):
    s1, m1 = lif(x_t @ w1.T + b1, m1)     # 129 -> 1000
    s2, m2 = lif(s1 @ w2.T + b2, m2)      # 1000 -> 1000
    s3, m3 = lif(s2 @ w3.T + b3, m3)      # 1000 -> 20
    s4, m4 = lif(s3 @ w4.T + b4, m4)      # 20 -> 10
with lif: reset = (m > 1); m' = 0.95*m + cur - reset; spk = (m' > 1)
Outputs: (s4, m4) per step -> each [81, 4096, 10].

Strategy:
- Data parallel over 8 NeuronCores: 512 batch rows per core; weights
  replicated; no cross-device traffic.
- Hidden-on-partition, batch-on-free layout: weights are the stationary
  lhsT, spikes/x the moving rhs; the 81-step scan needs no transposes.
- Layer 1 (129->1000): fp32r (12-bit) hi/lo cross terms wh@xh + wh@xl +
  wl@xh + a K=5 combo matmul (last feature + 24-bit bias rows), 1 PE
  cycle/column each. The reset subtraction rides the psum group as a -I
  diag matmul so no elementwise pass needs the previous spikes.
- Layer 2 (1000->1000), the dominant cost: 4-term fp8(e4m3) cascade in
  DoubleRow pairs at 0.5 cycles/column and 252 contraction rows per pass
  (verified bit-for-bit on HW; effective weight error ~6e-8, measured
  zero output spike flips). Group 0 lhsT pairs (t0, t1*2^4) multiply rhs
  pairs (s1, s1*2^-4) packed by two ACT converts per tile; group 1 pairs
  (t2*2^12, t3*2^16) reuse the same rhs into a second psum, merged with
  *2^-12 in the membrane update.
- Layers 3/4: single 12-bit fp32r terms (measured flip-free) with 24-bit
  hi/lo bias rows; layer-2 spikes arrive as {0,2} so w3 is halved.
- LIF membrane updates are beta-fused pre-updates: one stt per tile
  m = beta*m + psum (+ the psum's embedded resets/biases), then the spike
  threshold; layer-2 subtracts -0.5*u_old in a second stt. Elementwise
  work is spread so DVE keeps the psum ops, Pool the thresholds/subs, and
  ACT the fp8 packs; per-tile chains pipeline across engines.
- Software pipelining: layer-1 psums + LIF + packs for step t+1 run inside
  iteration t; layer 4 of step t runs inside iteration t+1.
"""

import os
import sys

import numpy as np

for _p in ("/opt/trn_rl_repo", "/root/.axon_site/_ro/trn_rl_repo"):
    if os.path.isdir(_p) and _p not in sys.path:
        sys.path.insert(0, _p)

import concourse.bacc as bacc
import concourse.bass as bass
import concourse.mybir as mybir
import concourse.tile as tile
from concourse.bass_utils import run_bass_kernel_spmd
from concourse.tile_rust import add_dep_helper

# Problem constants (hardcoded; kernel.py must be self-contained).
T = 81          # time steps
F = 129         # input features per step
H = 1000        # hidden units (layers 1, 2)
HT = 125        # hidden tile rows  (H = 8 * 125)
NH = 8          # number of hidden tiles
H3 = 20         # layer-3 units
H4 = 10         # output units
BATCH = 4096
NCORES = 8
B = BATCH // NCORES   # 512 batch rows per core
BETA = 0.95
THRESH = 1.0
XR = 2 * 128 + 5      # x_aug rows: xh[0:128], xl[0:128], 5 combo rows

F32 = mybir.dt.float32
F32R = mybir.dt.float32r
F8 = mybir.dt.float8e4
AOP = mybir.AluOpType
DR = mybir.MatmulPerfMode.DoubleRow
MT = 128              # layer-2 output tile rows, padded to the 128 the
                      # dual-fp8 ldweights ISA check requires (H = 8*125)
HP = NH * MT          # padded layer-2 output columns in the fp8 lhsT


def build_bass():
    # Bacc (not raw Bass): its compile() runs generate_event_semaphores /
    # move_matmul_waits_to_ldweights, required because TRN2 Matmult
    # instructions can carry at most one sync wait.
    nc = bacc.Bacc(trn_type="TRN2", target_bir_lowering=False)

    x_d = nc.dram_tensor("x_aug", [T, XR, B], F32R, kind="ExternalInput")
    w1h_d = nc.dram_tensor("w1h", [128, H], F32R, kind="ExternalInput")
    w1l_d = nc.dram_tensor("w1l", [128, H], F32R, kind="ExternalInput")
    w1c_d = nc.dram_tensor("w1c", [5, H], F32R, kind="ExternalInput")
    # -identity: folds the layer-1 reset subtraction into the psum group,
    # keeping the Pool engine out of the per-step LIF critical chain
    l1d_d = nc.dram_tensor("l1d", [HT, HT], F32R, kind="ExternalInput")
    # layer-2 weights as a 4-term fp8(e4m3) cascade in DoubleRow pairs:
    # group 0 lhsT pairs (t0, t1*2^4) with rhs pairs (s, s*2^-4); group 1
    # pairs (t2*2^12, t3*2^16) share the same rhs, psum merged with *2^-12.
    # Effective weight error ~6e-8 (measured zero output spike flips), at
    # 0.5 PE cycles/column and 252 contraction rows per pass.
    w2p_d = nc.dram_tensor("w2p", [2, NH, HT + 2, 2, HP], F8,
                           kind="ExternalInput")
    # layer-3 weights: single 12-bit fp32r term (measured: zero output spike
    # flips end-to-end), 8 matmuls into one [20, B] psum, rhs is s2 directly.
    # Bias stays 24-bit via hi+lo rows 125/126 of the last k-tile (the s2
    # tile carries two ones-rows there).
    w3r_d = nc.dram_tensor("w3r", [NH, HT + 2, H3], F32R, kind="ExternalInput")
    # layer-4 lhsT: rows 0..19 w4 (12-bit, also flip-free), rows 20/21 bias
    # hi/lo riding ones-rows of the s3 tile
    w4c_d = nc.dram_tensor("w4c", [22, H4], F32R, kind="ExternalInput")
    outs_d = nc.dram_tensor("out_s", [T, H4, B], F32, kind="ExternalOutput")
    outm_d = nc.dram_tensor("out_m", [T, H4, B], F32, kind="ExternalOutput")

    with tile.TileContext(nc) as tc:
        with (
            tc.tile_pool(name="pers", bufs=1) as pers,
            tc.tile_pool(name="xpool", bufs=3) as xpool,
            tc.tile_pool(name="ps1", bufs=2, space="PSUM") as ps1,
            tc.tile_pool(name="ps2", bufs=2, space="PSUM") as ps2,
            tc.tile_pool(name="ps3", bufs=1, space="PSUM") as ps3,
            tc.tile_pool(name="ps4", bufs=1, space="PSUM") as ps4,
        ):
            # ---- persistent SBUF tensors ----
            w1h = pers.tile([128, H], F32R, tag="w1h")
            w1l = pers.tile([128, H], F32R, tag="w1l")
            w1c = pers.tile([5, H], F32R, tag="w1c")
            l1d = pers.tile([HT, HT], F32R, tag="l1d")
            w2p = pers.tile([HT + 2, 2, NH, 2, HP], F8, tag="w2p")
            s1p = pers.tile([HT + 2, 2, NH * B], F8, tag="s1p")
            w3r = pers.tile([HT + 2, NH * H3], F32R, tag="w3r")  # [127, 160]
            w4c = pers.tile([22, H4], F32R, tag="w4c")
            m1 = pers.tile([HT, NH * B], F32, tag="m1")          # [125, 4096]
            m2 = pers.tile([HT, NH * B], F32, tag="m2")
            m3 = pers.tile([H3, B], F32, tag="m3")               # [20, 512]
            m4 = pers.tile([H4, B], F32, tag="m4")               # [10, 512]
            s1 = pers.tile([HT + 1, NH * B], F32R, tag="s1")     # [126, 4096]
            s2 = pers.tile([HT + 2, NH * B], F32R, tag="s2")     # [127, 4096]
            s3 = pers.tile([22, B], F32R, tag="s3")
            s4 = pers.tile([H4, B], F32, tag="s4")

            # fp32 views of the fp32r spike tiles for elementwise consumers
            s1f = s1[:].bitcast(F32)
            s2f = s2[:].bitcast(F32)
            s3f = s3[:].bitcast(F32)

            # ---- weight loads (layer-1 weights + x(0) first: they gate
            # step 0; the bulk w2/w3 transfers follow) ----
            def load_x(t):
                xh = xpool.tile([128, B], F32R, tag="xh", name="xh")
                xl = xpool.tile([128, B], F32R, tag="xl", name="xl")
                xc = xpool.tile([5, B], F32R, tag="xc", name="xc")
                nc.sync.dma_start(xh[:], x_d[t, 0:128, :])
                nc.sync.dma_start(xl[:], x_d[t, 128:256, :])
                nc.sync.dma_start(xc[:], x_d[t, 256:261, :])
                return xh, xl, xc

            w1dmas = []
            for sb, dr in [(w1h, w1h_d), (w1l, w1l_d), (w1c, w1c_d),
                           (l1d, l1d_d)]:
                w1dmas.append(nc.sync.dma_start(sb[:], dr[:]))
            x0 = load_x(0)
            wdmas = [nc.sync.dma_start(w4c[:], w4c_d[:])]
            for k in range(NH):
                for g in range(2):
                    wdmas.append(
                        nc.sync.dma_start(w2p[:, g, k, :, :], w2p_d[g, k]))
                wdmas.append(nc.sync.dma_start(
                    w3r[:, k * H3:(k + 1) * H3], w3r_d[k]))

            # Matmult instructions can carry at most ONE sync wait in the
            # TRN2 ISA (fp32/fp32r fuse the weight load into the matmul), so
            # have PE nops absorb the weight-DMA waits before any matmul.
            # Layer-1 absorbers go before the prologue; the rest only need to
            # precede the main loop's first layer-2/3/4 matmuls.
            def absorb(dmas):
                nops = []
                for d in dmas:
                    nop = nc.tensor.nop(nofuse=True)
                    add_dep_helper(nop.ins, d.ins, sync=True,
                                   reason="absorb weight-DMA wait on PE")
                    nops.append(nop)
                return nops

            absorbers = absorb(w1dmas)

            # ---- state init ----
            nc.vector.memset(m1[:], 0.0)
            nc.vector.memset(m2[:], 0.0)
            nc.gpsimd.memset(m3[:], 0.0)
            nc.gpsimd.memset(m4[:], 0.0)
            nc.gpsimd.memset(s4[:], 0.0)
            # ones rows feeding the bias fold (k-tile 7 / layer-4 rhs).
            # Engine ops need partition bases in {0,32,64,96}, so memset a
            # wider aligned region; all rows except the last are overwritten
            # by the per-step spike writes before any matmul reads them.
            nc.vector.memset(s1f[96:HT + 1, (NH - 1) * B:], 1.0)
            nc.vector.memset(s2f[96:HT + 2, (NH - 1) * B:], 1.0)
            # pre-update structure reads the previous step's spikes before
            # computing new ones, so spike rows need a zero init AFTER the
            # ones-row memsets above (row 125+ keeps the bias ones)
            nc.vector.memset(s1f[0:HT, :], 0.0)
            nc.vector.memset(s2f[0:HT, :], 0.0)
            nc.gpsimd.memset(s3f[:, :], 1.0)   # rows 20/21 stay as ones rows
            nc.gpsimd.memset(s3f[0:H3, :], 0.0)
            # fp8 rhs pack: row 125 holds the (1, 2^-4) constant pair that
            # multiplies the bias rows of the last k-tile; the ACT pack
            # copies overwrite rows 0..124 every step
            nc.vector.memset(s1p[96:HT + 2, 0, :], 1.0)
            nc.vector.memset(s1p[96:HT + 2, 1, :], 0.0625)

            def l1_block(xh, xl, xc):
                """Layer-1 psums + LIF + fp8 packs for one step, per tile.

                Per-tile chain: psum (incl -I*s1_old reset) -> m1 = b*m1+p
                (DVE stt) -> spike (Pool is_gt) -> fp8 packs (ACT), so each
                engine touches a tile once and tiles pipeline."""
                first_mm = None
                for h in range(NH):
                    p1 = ps1.tile([HT, B], F32, tag="p1")
                    c0 = h * HT
                    cols = slice(h * B, (h + 1) * B)
                    mm = nc.tensor.matmul(p1[:], l1d[:], s1[0:HT, cols],
                                          start=True, stop=False)
                    if first_mm is None:
                        first_mm = mm
                    nc.tensor.matmul(p1[:], w1h[:, c0:c0 + HT], xh[:],
                                     start=False, stop=False)
                    nc.tensor.matmul(p1[:], w1h[:, c0:c0 + HT], xl[:],
                                     start=False, stop=False)
                    nc.tensor.matmul(p1[:], w1l[:, c0:c0 + HT], xh[:],
                                     start=False, stop=False)
                    nc.tensor.matmul(p1[:], w1c[:, c0:c0 + HT], xc[:],
                                     start=False, stop=True)
                    nc.vector.scalar_tensor_tensor(m1[:, cols], m1[:, cols],
                                                   BETA, p1[:],
                                                   AOP.mult, AOP.add)
                    nc.gpsimd.tensor_scalar(s1[0:HT, cols], m1[:, cols],
                                            THRESH, None, AOP.is_gt)
                    nc.scalar.copy(s1p[0:HT, 0, cols], s1f[0:HT, cols])
                    nc.scalar.activation(s1p[0:HT, 1, cols], s1f[0:HT, cols],
                                         mybir.ActivationFunctionType.Copy,
                                         scale=0.0625)
                return first_mm

            # ---- prologue: step 0 layer-1 ----
            first_mm = l1_block(*x0)
            for nop in absorbers:
                add_dep_helper(first_mm.ins, nop.ins, sync=False,
                               reason="keep absorbers before first matmul")

            # absorb the remaining weight DMAs before the main loop's
            # layer-2/3/4 matmuls
            late_absorbers = absorb(wdmas)

            def l4_block(t):
                """Layer 4 for step t + LIF + output DMAs."""
                p4 = ps4.tile([H4, B], F32, tag="p4")
                # -I diag carries the reset subtraction (reads s4 of step
                # t-1 before this step's is_gt rewrites it)
                nc.tensor.matmul(p4[:], l1d[0:H4, 0:H4],
                                 s4[:].bitcast(F32R),
                                 start=True, stop=False)
                nc.tensor.matmul(p4[:], w4c[:], s3[:], start=False, stop=True)
                nc.vector.scalar_tensor_tensor(m4[:], m4[:], BETA, p4[:],
                                               AOP.mult, AOP.add)
                nc.sync.dma_start(outm_d[t], m4[:])
                nc.gpsimd.tensor_scalar(s4[:].bitcast(F32R), m4[:], THRESH,
                                        None, AOP.is_gt)
                nc.sync.dma_start(outs_d[t], s4[:])

            # ---- main loop over steps ----
            for i in range(T):
                if i < T - 1:
                    xh, xl, xc = load_x(i + 1)

                # layer 2 of step i: fp8 DoubleRow, two psum scale-groups;
                # each pass consumes 252 contraction rows at 256 PE cycles
                for h in range(NH):
                    c0 = h * MT
                    pA = ps2.tile([MT, B], F32, tag="pA")
                    pB = ps2.tile([MT, B], F32, tag="pB")
                    for g, pX in ((0, pA), (1, pB)):
                        for j in range(NH):
                            k = (h + j) % NH
                            mm2 = nc.tensor.matmul(
                                pX[:],
                                w2p[0:HT + 2, g, k, :, c0:c0 + MT],
                                s1p[0:HT + 2, :, k * B:(k + 1) * B],
                                start=(j == 0),
                                stop=(j == NH - 1),
                                perf_mode=DR)
                            if i == 0 and h == 0 and j == 0 and g == 0:
                                for nop in late_absorbers:
                                    add_dep_helper(
                                        mm2.ins, nop.ins, sync=False,
                                        reason="absorbers before first L2 mm")
                    cols = slice(h * B, (h + 1) * B)
                    # m2 = beta*m2 + psumA + psumB*2^-12 - u_old/2 with
                    # u = 2*(m2>1); layer-3 weights are halved host-side.
                    nc.vector.scalar_tensor_tensor(m2[:, cols], m2[:, cols],
                                                   BETA, pA[0:HT, :],
                                                   AOP.mult, AOP.add)
                    nc.vector.scalar_tensor_tensor(m2[:, cols], pB[0:HT, :],
                                                   2.0 ** -12, m2[:, cols],
                                                   AOP.mult, AOP.add)
                    nc.vector.scalar_tensor_tensor(m2[:, cols],
                                                   s2f[0:HT, cols], -0.5,
                                                   m2[:, cols],
                                                   AOP.mult, AOP.add)
                    # spikes as {0, 2}: exact is_gt (ACT Sign rounds near
                    # the threshold), then *2 in the same instruction; the
                    # halved layer-3 weights and the -0.5 subtract coeff
                    # make u/2 the effective 0/1 spike
                    nc.gpsimd.tensor_scalar(s2[0:HT, cols], m2[:, cols],
                                            THRESH, 2.0, AOP.is_gt, AOP.mult)

                # layer 4 of step i-1 (deferred so spk3 is long ready)
                if i > 0:
                    l4_block(i - 1)

                # layer-1 psums + LIF + packs for step i+1
                if i < T - 1:
                    l1_block(xh, xl, xc)

                # layer 3 of step i: single 12-bit fp32r term (weights
                # pre-halved for the +-1 spike convention), 8 matmuls into
                # one [20, B] psum
                p3 = ps3.tile([H3, B], F32, tag="p3")
                # -I diag: reset subtraction reads s3 of step i-1 (after
                # l4_block(i-1) consumed it, before this step's is_gt)
                nc.tensor.matmul(p3[:], l1d[0:H3, 0:H3], s3[0:H3, :],
                                 start=True, stop=False)
                for k in range(NH):
                    kk = HT + 2 if k == NH - 1 else HT
                    nc.tensor.matmul(
                        p3[:],
                        w3r[0:kk, k * H3:(k + 1) * H3],
                        s2[0:kk, k * B:(k + 1) * B],
                        start=False,
                        stop=(k == NH - 1))
                nc.vector.scalar_tensor_tensor(m3[:], m3[:], BETA, p3[:],
                                               AOP.mult, AOP.add)
                nc.gpsimd.tensor_scalar(s3[0:H3, :], m3[:], THRESH, None,
                                        AOP.is_gt)

            # ---- epilogue ----
            l4_block(T - 1)

    nc.compile()
    return nc


_CACHE = {}


def _get_nc():
    if "nc" not in _CACHE:
        _CACHE["nc"] = build_bass()
    return _CACHE["nc"]


def _rne12(a):
    """Round fp32 to 12 significand bits (the fp32r grid), RNE —
    bit-identical to the device's fp32r rounding."""
    drop = np.uint64(12)
    u = np.ascontiguousarray(a, np.float32).view(np.uint32).astype(np.uint64)
    half = np.uint64(1 << 11)
    lsb = (u >> drop) & np.uint64(1)
    u2 = ((u + half - np.uint64(1) + lsb) >> drop << drop)
    return u2.astype(np.uint32).view(np.float32).reshape(a.shape)


def _hilo(a):
    hi = _rne12(a)
    lo = _rne12(np.asarray(a, np.float32) - hi)
    return hi, lo


def _prep_inputs(x, w1, b1, w2, b2, w3, b3, w4, b4):
    x = np.ascontiguousarray(x, np.float32)
    # xs[t, f, b_global]; step t of the reference reads x[:, f*T + t]
    xt = np.ascontiguousarray(
        np.transpose(x.reshape(BATCH, F, T), (2, 1, 0)))   # [T, F, BATCH]
    xth, xtl = _hilo(xt)

    w1T = np.ascontiguousarray(w1.T.astype(np.float32))    # [129, 1000]
    w1h, w1l = _hilo(w1T[:128])
    whL, wlL = _hilo(w1T[128])
    b1h, b1l = _hilo(b1.astype(np.float32))
    w1c = np.stack([whL, whL, wlL, b1h, b1l])              # [5, 1000]

    # layer-2: 4-term e4m3 cascade of [w2.T; b2] -> DoubleRow-paired lhsT.
    # Terms t0..t3 at effective scales 1, 2^-4, 2^-12, 2^-16; groups
    # (t0, t1*2^4) and (t2*2^12, t3*2^16) pair with rhs (s, s*2^-4).
    import ml_dtypes
    e4 = ml_dtypes.float8_e4m3

    def q8(a):
        return a.astype(e4).astype(np.float32)

    wb2 = np.concatenate([np.ascontiguousarray(w2.T.astype(np.float32)),
                          b2.astype(np.float32)[None, :]])   # [1001, 1000]
    t0 = q8(wb2)
    r = wb2 - t0
    t1 = q8(r * 2.0 ** 4)
    r = r - t1 * 2.0 ** -4
    t2 = q8(r * 2.0 ** 12)
    r = r - t2 * 2.0 ** -12
    t3 = q8(r * 2.0 ** 16)
    w2p = np.zeros((2, NH, HT + 2, 2, NH * 128), e4)
    terms = ((t0, t1), (t2, t3))
    for g in range(2):
        for k in range(NH):
            for i2 in range(2):
                src = terms[g][i2][k * HT:(k + 1) * HT]      # [125, 1000]
                for h in range(NH):
                    w2p[g, k, :HT, i2, h * 128:h * 128 + HT] = (
                        src[:, h * HT:(h + 1) * HT])
        for i2 in range(2):
            brow = terms[g][i2][H]                           # bias row
            for h in range(NH):
                w2p[g, NH - 1, HT, i2, h * 128:h * 128 + HT] = (
                    brow[h * HT:(h + 1) * HT])

    # layer-3: single 12-bit term; layer-2 spikes arrive as {0,2}, so the
    # weights are halved (exact) and the bias stays plain 24-bit hi/lo
    w3q = _rne12(np.ascontiguousarray(w3.T.astype(np.float32)) * 0.5)
    b3h, b3l = _hilo(b3.astype(np.float32))
    w3r = np.zeros((NH, HT + 2, H3), np.float32)
    for k in range(NH):
        w3r[k, :HT] = w3q[k * HT:(k + 1) * HT]
    w3r[NH - 1, HT] = b3h
    w3r[NH - 1, HT + 1] = b3l

    l1d = np.ascontiguousarray(-np.eye(HT, dtype=np.float32))

    b4h, b4l = _hilo(b4.astype(np.float32))
    w4c = np.zeros((22, H4), np.float32)
    w4c[0:20] = _rne12(w4.T.astype(np.float32))
    w4c[20] = b4h
    w4c[21] = b4l

    in_maps = []
    for c in range(NCORES):
        xc = np.empty((T, XR, B), np.float32)
        xc[:, 0:128, :] = xth[:, 0:128, c * B:(c + 1) * B]
        xc[:, 128:256, :] = xtl[:, 0:128, c * B:(c + 1) * B]
        xc[:, 256, :] = xth[:, 128, c * B:(c + 1) * B]
        xc[:, 257, :] = xtl[:, 128, c * B:(c + 1) * B]
        xc[:, 258, :] = xth[:, 128, c * B:(c + 1) * B]
        xc[:, 259, :] = 1.0
        xc[:, 260, :] = 1.0
        in_maps.append({
            "x_aug": xc, "w1h": w1h, "w1l": w1l, "w1c": w1c, "l1d": l1d,
            "w2p": w2p, "w3r": w3r, "w4c": w4c,
        })
    return in_maps


def _gather(results):
    spk = np.concatenate(
        [np.transpose(r["out_s"], (0, 2, 1)) for r in results], axis=1)
    mem = np.concatenate(
        [np.transpose(r["out_m"], (0, 2, 1)) for r in results], axis=1)
    return spk, mem


def kernel(x, w1, b1, w2, b2, w3, b3, w4, b4, _trace=False, _trace_kwargs=None):
    # accept numpy or jax arrays, any float dtype
    x, w1, b1, w2, b2, w3, b3, w4, b4 = (
        np.asarray(a, dtype=np.float32)
        for a in (x, w1, b1, w2, b2, w3, b3, w4, b4))
    nc = _get_nc()
    in_maps = _prep_inputs(x, w1, b1, w2, b2, w3, b3, w4, b4)
    res = run_bass_kernel_spmd(
        nc, in_maps, core_ids=list(range(NCORES)),
        trace=_trace, **(_trace_kwargs or {}))
    out = _gather(res.results)
    if _trace:
        return out, res
    return out



# revision 56
# speedup vs baseline: 1.0854x; 1.0146x over previous
"""Trainium2 Bass kernel for the AudNet 4-layer LIF spiking network.

Reference computation (per time step t of 81, batch 4096):
    s1, m1 = lif(x_t @ w1.T + b1, m1)     # 129 -> 1000
    s2, m2 = lif(s1 @ w2.T + b2, m2)      # 1000 -> 1000
    s3, m3 = lif(s2 @ w3.T + b3, m3)      # 1000 -> 20
    s4, m4 = lif(s3 @ w4.T + b4, m4)      # 20 -> 10
with lif: reset = (m > 1); m' = 0.95*m + cur - reset; spk = (m' > 1)
Outputs: (s4, m4) per step -> each [81, 4096, 10].

Strategy:
- Data parallel over 8 NeuronCores: 512 batch rows per core; weights
  replicated; no cross-device traffic.
- Hidden-on-partition, batch-on-free layout: weights are the stationary
  lhsT, spikes/x the moving rhs; the 81-step scan needs no transposes.
- Layer 1 (129->1000): fp32r (12-bit) hi/lo cross terms wh@xh + wh@xl +
  wl@xh + a K=5 combo matmul (last feature + 24-bit bias rows), 1 PE
  cycle/column each. The reset subtraction rides the psum group as a -I
  diag matmul so no elementwise pass needs the previous spikes.
- Layer 2 (1000->1000), the dominant cost: 4-term fp8(e4m3) cascade in
  DoubleRow pairs at 0.5 cycles/column and 252 contraction rows per pass
  (verified bit-for-bit on HW; effective weight error ~6e-8, measured
  zero output spike flips). Group 0 lhsT pairs (t0, t1*2^4) multiply rhs
  pairs (s1, s1*2^-4) packed by two ACT converts per tile; group 1 pairs
  (t2*2^12, t3*2^16) reuse the same rhs into a second psum, merged with
  *2^-12 in the membrane update.
- Layers 3/4: single 12-bit fp32r terms (measured flip-free) with 24-bit
  hi/lo bias rows; layer-2 spikes arrive as {0,2} so w3 is halved.
- LIF membrane updates are beta-fused pre-updates: one stt per tile
  m = beta*m + psum (+ the psum's embedded resets/biases), then the spike
  threshold; layer-2 subtracts -0.5*u_old in a second stt. Elementwise
  work is spread so DVE keeps the psum ops, Pool the thresholds/subs, and
  ACT the fp8 packs; per-tile chains pipeline across engines.
- Software pipelining: layer-1 psums + LIF + packs for step t+1 run inside
  iteration t; layer 4 of step t runs inside iteration t+1.
"""

import os
import sys

import numpy as np

for _p in ("/opt/trn_rl_repo", "/root/.axon_site/_ro/trn_rl_repo"):
    if os.path.isdir(_p) and _p not in sys.path:
        sys.path.insert(0, _p)

import concourse.bacc as bacc
import concourse.bass as bass
import concourse.mybir as mybir
import concourse.tile as tile
from concourse.bass_utils import run_bass_kernel_spmd
from concourse.tile_rust import add_dep_helper

# Problem constants (hardcoded; kernel.py must be self-contained).
T = 81          # time steps
F = 129         # input features per step
H = 1000        # hidden units (layers 1, 2)
HT = 125        # hidden tile rows  (H = 8 * 125)
NH = 8          # number of hidden tiles
H3 = 20         # layer-3 units
H4 = 10         # output units
BATCH = 4096
NCORES = 8
B = BATCH // NCORES   # 512 batch rows per core
BETA = 0.95
THRESH = 1.0
XR = 2 * 128 + 5      # x_aug rows: xh[0:128], xl[0:128], 5 combo rows

F32 = mybir.dt.float32
F32R = mybir.dt.float32r
F8 = mybir.dt.float8e4
AOP = mybir.AluOpType
DR = mybir.MatmulPerfMode.DoubleRow
MT = 128              # layer-2 output tile rows, padded to the 128 the
                      # dual-fp8 ldweights ISA check requires (H = 8*125)
HP = NH * MT          # padded layer-2 output columns in the fp8 lhsT


def build_bass():
    # Bacc (not raw Bass): its compile() runs generate_event_semaphores /
    # move_matmul_waits_to_ldweights, required because TRN2 Matmult
    # instructions can carry at most one sync wait.
    nc = bacc.Bacc(trn_type="TRN2", target_bir_lowering=False)

    x_d = nc.dram_tensor("x_aug", [T, XR, B], F32R, kind="ExternalInput")
    w1h_d = nc.dram_tensor("w1h", [128, H], F32R, kind="ExternalInput")
    w1l_d = nc.dram_tensor("w1l", [128, H], F32R, kind="ExternalInput")
    w1c_d = nc.dram_tensor("w1c", [5, H], F32R, kind="ExternalInput")
    # -identity: folds the layer-1 reset subtraction into the psum group,
    # keeping the Pool engine out of the per-step LIF critical chain
    l1d_d = nc.dram_tensor("l1d", [HT, HT], F32R, kind="ExternalInput")
    # layer-2 weights as a 4-term fp8(e4m3) cascade in DoubleRow pairs:
    # group 0 lhsT pairs (t0, t1*2^4) with rhs pairs (s, s*2^-4); group 1
    # pairs (t2*2^12, t3*2^16) share the same rhs, psum merged with *2^-12.
    # Effective weight error ~6e-8 (measured zero output spike flips), at
    # 0.5 PE cycles/column and 252 contraction rows per pass.
    w2p_d = nc.dram_tensor("w2p", [2, NH, HT + 2, 2, HP], F8,
                           kind="ExternalInput")
    # layer-3 weights: single 12-bit fp32r term (measured: zero output spike
    # flips end-to-end), 8 matmuls into one [20, B] psum, rhs is s2 directly.
    # Bias stays 24-bit via hi+lo rows 125/126 of the last k-tile (the s2
    # tile carries two ones-rows there).
    w3r_d = nc.dram_tensor("w3r", [NH, HT + 2, H3], F32R, kind="ExternalInput")
    # layer-4 lhsT: rows 0..19 w4 (12-bit, also flip-free), rows 20/21 bias
    # hi/lo riding ones-rows of the s3 tile
    w4c_d = nc.dram_tensor("w4c", [22, H4], F32R, kind="ExternalInput")
    outs_d = nc.dram_tensor("out_s", [T, H4, B], F32, kind="ExternalOutput")
    outm_d = nc.dram_tensor("out_m", [T, H4, B], F32, kind="ExternalOutput")

    with tile.TileContext(nc) as tc:
        with (
            tc.tile_pool(name="pers", bufs=1) as pers,
            tc.tile_pool(name="xpool", bufs=3) as xpool,
            tc.tile_pool(name="ps1", bufs=2, space="PSUM") as ps1,
            tc.tile_pool(name="ps2", bufs=2, space="PSUM") as ps2,
            tc.tile_pool(name="ps3", bufs=1, space="PSUM") as ps3,
            tc.tile_pool(name="ps4", bufs=1, space="PSUM") as ps4,
        ):
            # ---- persistent SBUF tensors ----
            w1h = pers.tile([128, H], F32R, tag="w1h")
            w1l = pers.tile([128, H], F32R, tag="w1l")
            w1c = pers.tile([5, H], F32R, tag="w1c")
            l1d = pers.tile([HT, HT], F32R, tag="l1d")
            w2p = pers.tile([HT + 2, 2, NH, 2, HP], F8, tag="w2p")
            s1p = pers.tile([HT + 2, 2, NH * B], F8, tag="s1p")
            w3r = pers.tile([HT + 2, NH * H3], F32R, tag="w3r")  # [127, 160]
            w4c = pers.tile([22, H4], F32R, tag="w4c")
            m1 = pers.tile([HT, NH * B], F32, tag="m1")          # [125, 4096]
            m2 = pers.tile([HT, NH * B], F32, tag="m2")
            m3 = pers.tile([H3, B], F32, tag="m3")               # [20, 512]
            m4 = pers.tile([H4, B], F32, tag="m4")               # [10, 512]
            s1 = pers.tile([HT + 1, NH * B], F32R, tag="s1")     # [126, 4096]
            s2 = pers.tile([HT + 2, NH * B], F32R, tag="s2")     # [127, 4096]
            s3 = pers.tile([22, B], F32R, tag="s3")
            s4 = pers.tile([H4, B], F32, tag="s4")

            # fp32 views of the fp32r spike tiles for elementwise consumers
            s1f = s1[:].bitcast(F32)
            s2f = s2[:].bitcast(F32)
            s3f = s3[:].bitcast(F32)

            # ---- weight loads (layer-1 weights + x(0) first: they gate
            # step 0; the bulk w2/w3 transfers follow) ----
            def load_x(t):
                xh = xpool.tile([128, B], F32R, tag="xh", name="xh")
                xl = xpool.tile([128, B], F32R, tag="xl", name="xl")
                xc = xpool.tile([5, B], F32R, tag="xc", name="xc")
                nc.sync.dma_start(xh[:], x_d[t, 0:128, :])
                nc.sync.dma_start(xl[:], x_d[t, 128:256, :])
                nc.sync.dma_start(xc[:], x_d[t, 256:261, :])
                return xh, xl, xc

            w1dmas = []
            for sb, dr in [(w1h, w1h_d), (w1l, w1l_d), (w1c, w1c_d),
                           (l1d, l1d_d)]:
                w1dmas.append(nc.sync.dma_start(sb[:], dr[:]))
            x0 = load_x(0)
            wdmas = [nc.sync.dma_start(w4c[:], w4c_d[:])]
            for k in range(NH):
                for g in range(2):
                    wdmas.append(
                        nc.sync.dma_start(w2p[:, g, k, :, :], w2p_d[g, k]))
                wdmas.append(nc.sync.dma_start(
                    w3r[:, k * H3:(k + 1) * H3], w3r_d[k]))

            # Matmult instructions can carry at most ONE sync wait in the
            # TRN2 ISA (fp32/fp32r fuse the weight load into the matmul), so
            # have PE nops absorb the weight-DMA waits before any matmul.
            # Layer-1 absorbers go before the prologue; the rest only need to
            # precede the main loop's first layer-2/3/4 matmuls.
            def absorb(dmas):
                nops = []
                for d in dmas:
                    nop = nc.tensor.nop(nofuse=True)
                    add_dep_helper(nop.ins, d.ins, sync=True,
                                   reason="absorb weight-DMA wait on PE")
                    nops.append(nop)
                return nops

            absorbers = absorb(w1dmas)

            # ---- state init ----
            nc.vector.memset(m1[:], 0.0)
            nc.vector.memset(m2[:], 0.0)
            nc.gpsimd.memset(m3[:], 0.0)
            nc.gpsimd.memset(m4[:], 0.0)
            nc.gpsimd.memset(s4[:], 0.0)
            # ones rows feeding the bias fold (k-tile 7 / layer-4 rhs).
            # Engine ops need partition bases in {0,32,64,96}, so memset a
            # wider aligned region; all rows except the last are overwritten
            # by the per-step spike writes before any matmul reads them.
            nc.vector.memset(s1f[96:HT + 1, (NH - 1) * B:], 1.0)
            nc.vector.memset(s2f[96:HT + 2, (NH - 1) * B:], 1.0)
            # pre-update structure reads the previous step's spikes before
            # computing new ones, so spike rows need a zero init AFTER the
            # ones-row memsets above (row 125+ keeps the bias ones)
            nc.vector.memset(s1f[0:HT, :], 0.0)
            nc.vector.memset(s2f[0:HT, :], 0.0)
            nc.gpsimd.memset(s3f[:, :], 1.0)   # rows 20/21 stay as ones rows
            nc.gpsimd.memset(s3f[0:H3, :], 0.0)
            # fp8 rhs pack: row 125 holds the (1, 2^-4) constant pair that
            # multiplies the bias rows of the last k-tile; the ACT pack
            # copies overwrite rows 0..124 every step
            nc.vector.memset(s1p[96:HT + 2, 0, :], 1.0)
            nc.vector.memset(s1p[96:HT + 2, 1, :], 0.0625)

            def l1_block(xh, xl, xc):
                """Layer-1 psums + LIF + fp8 packs for one step, per tile.

                Per-tile chain: psum (incl -I*s1_old reset) -> m1 = b*m1+p
                (DVE stt) -> spike (Pool is_gt) -> fp8 packs (ACT), so each
                engine touches a tile once and tiles pipeline."""
                first_mm = None
                for h in range(NH):
                    p1 = ps1.tile([HT, B], F32, tag="p1")
                    c0 = h * HT
                    cols = slice(h * B, (h + 1) * B)
                    mm = nc.tensor.matmul(p1[:], l1d[:], s1[0:HT, cols],
                                          start=True, stop=False)
                    if first_mm is None:
                        first_mm = mm
                    nc.tensor.matmul(p1[:], w1h[:, c0:c0 + HT], xh[:],
                                     start=False, stop=False)
                    nc.tensor.matmul(p1[:], w1h[:, c0:c0 + HT], xl[:],
                                     start=False, stop=False)
                    nc.tensor.matmul(p1[:], w1l[:, c0:c0 + HT], xh[:],
                                     start=False, stop=False)
                    nc.tensor.matmul(p1[:], w1c[:, c0:c0 + HT], xc[:],
                                     start=False, stop=True)
                    nc.vector.scalar_tensor_tensor(m1[:, cols], m1[:, cols],
                                                   BETA, p1[:],
                                                   AOP.mult, AOP.add)
                    nc.gpsimd.tensor_scalar(s1[0:HT, cols], m1[:, cols],
                                            THRESH, None, AOP.is_gt)
                    nc.scalar.copy(s1p[0:HT, 0, cols], s1f[0:HT, cols])
                    nc.scalar.activation(s1p[0:HT, 1, cols], s1f[0:HT, cols],
                                         mybir.ActivationFunctionType.Copy,
                                         scale=0.0625)
                return first_mm

            # ---- prologue: step 0 layer-1 ----
            first_mm = l1_block(*x0)
            for nop in absorbers:
                add_dep_helper(first_mm.ins, nop.ins, sync=False,
                               reason="keep absorbers before first matmul")

            # absorb the remaining weight DMAs before the main loop's
            # layer-2/3/4 matmuls
            late_absorbers = absorb(wdmas)

            def l4_block(t):
                """Layer 4 for step t + LIF + output DMAs."""
                p4 = ps4.tile([H4, B], F32, tag="p4")
                # -I diag carries the reset subtraction (reads s4 of step
                # t-1 before this step's is_gt rewrites it)
                nc.tensor.matmul(p4[:], l1d[0:H4, 0:H4],
                                 s4[:].bitcast(F32R),
                                 start=True, stop=False)
                nc.tensor.matmul(p4[:], w4c[:], s3[:], start=False, stop=True)
                nc.vector.scalar_tensor_tensor(m4[:], m4[:], BETA, p4[:],
                                               AOP.mult, AOP.add)
                nc.sync.dma_start(outm_d[t], m4[:])
                nc.vector.tensor_scalar(s4[:].bitcast(F32R), m4[:], THRESH,
                                        None, AOP.is_gt)
                nc.sync.dma_start(outs_d[t], s4[:])

            # ---- main loop over steps ----
            for i in range(T):
                if i < T - 1:
                    xh, xl, xc = load_x(i + 1)

                # layer 2 of step i: fp8 DoubleRow, two psum scale-groups;
                # each pass consumes 252 contraction rows at 256 PE cycles
                for h in range(NH):
                    c0 = h * MT
                    pA = ps2.tile([MT, B], F32, tag="pA")
                    pB = ps2.tile([MT, B], F32, tag="pB")
                    for g, pX in ((0, pA), (1, pB)):
                        for j in range(NH):
                            k = (h + j) % NH
                            mm2 = nc.tensor.matmul(
                                pX[:],
                                w2p[0:HT + 2, g, k, :, c0:c0 + MT],
                                s1p[0:HT + 2, :, k * B:(k + 1) * B],
                                start=(j == 0),
                                stop=(j == NH - 1),
                                perf_mode=DR)
                            if i == 0 and h == 0 and j == 0 and g == 0:
                                for nop in late_absorbers:
                                    add_dep_helper(
                                        mm2.ins, nop.ins, sync=False,
                                        reason="absorbers before first L2 mm")
                    cols = slice(h * B, (h + 1) * B)
                    # m2 = beta*m2 + psumA + psumB*2^-12 - u_old/2 with
                    # u = 2*(m2>1); layer-3 weights are halved host-side.
                    nc.vector.scalar_tensor_tensor(m2[:, cols], m2[:, cols],
                                                   BETA, pA[0:HT, :],
                                                   AOP.mult, AOP.add)
                    nc.vector.scalar_tensor_tensor(m2[:, cols], pB[0:HT, :],
                                                   2.0 ** -12, m2[:, cols],
                                                   AOP.mult, AOP.add)
                    nc.vector.scalar_tensor_tensor(m2[:, cols],
                                                   s2f[0:HT, cols], -0.5,
                                                   m2[:, cols],
                                                   AOP.mult, AOP.add)
                    # spikes as {0, 2}: exact is_gt (ACT Sign rounds near
                    # the threshold), then *2 in the same instruction; the
                    # halved layer-3 weights and the -0.5 subtract coeff
                    # make u/2 the effective 0/1 spike
                    nc.gpsimd.tensor_scalar(s2[0:HT, cols], m2[:, cols],
                                            THRESH, 2.0, AOP.is_gt, AOP.mult)

                # layer-1 psums + LIF + packs for step i+1
                if i < T - 1:
                    l1_block(xh, xl, xc)

                # layer 4 of step i-1, after the layer-1 block so its wait
                # on the s3 threshold chain is covered by layer-1 PE work
                if i > 0:
                    l4_block(i - 1)

                # layer 3 of step i: single 12-bit fp32r term (weights
                # pre-halved for the +-1 spike convention), 8 matmuls into
                # one [20, B] psum
                p3 = ps3.tile([H3, B], F32, tag="p3")
                # -I diag: reset subtraction reads s3 of step i-1 (after
                # l4_block(i-1) consumed it, before this step's is_gt)
                nc.tensor.matmul(p3[:], l1d[0:H3, 0:H3], s3[0:H3, :],
                                 start=True, stop=False)
                for k in range(NH):
                    kk = HT + 2 if k == NH - 1 else HT
                    nc.tensor.matmul(
                        p3[:],
                        w3r[0:kk, k * H3:(k + 1) * H3],
                        s2[0:kk, k * B:(k + 1) * B],
                        start=False,
                        stop=(k == NH - 1))
                nc.vector.scalar_tensor_tensor(m3[:], m3[:], BETA, p3[:],
                                               AOP.mult, AOP.add)
                nc.vector.tensor_scalar(s3[0:H3, :], m3[:], THRESH, None,
                                        AOP.is_gt)

            # ---- epilogue ----
            l4_block(T - 1)

    nc.compile()
    return nc


_CACHE = {}


def _get_nc():
    if "nc" not in _CACHE:
        _CACHE["nc"] = build_bass()
    return _CACHE["nc"]


def _rne12(a):
    """Round fp32 to 12 significand bits (the fp32r grid), RNE —
    bit-identical to the device's fp32r rounding."""
    drop = np.uint64(12)
    u = np.ascontiguousarray(a, np.float32).view(np.uint32).astype(np.uint64)
    half = np.uint64(1 << 11)
    lsb = (u >> drop) & np.uint64(1)
    u2 = ((u + half - np.uint64(1) + lsb) >> drop << drop)
    return u2.astype(np.uint32).view(np.float32).reshape(a.shape)


def _hilo(a):
    hi = _rne12(a)
    lo = _rne12(np.asarray(a, np.float32) - hi)
    return hi, lo


def _prep_inputs(x, w1, b1, w2, b2, w3, b3, w4, b4):
    x = np.ascontiguousarray(x, np.float32)
    # xs[t, f, b_global]; step t of the reference reads x[:, f*T + t]
    xt = np.ascontiguousarray(
        np.transpose(x.reshape(BATCH, F, T), (2, 1, 0)))   # [T, F, BATCH]
    xth, xtl = _hilo(xt)

    w1T = np.ascontiguousarray(w1.T.astype(np.float32))    # [129, 1000]
    w1h, w1l = _hilo(w1T[:128])
    whL, wlL = _hilo(w1T[128])
    b1h, b1l = _hilo(b1.astype(np.float32))
    w1c = np.stack([whL, whL, wlL, b1h, b1l])              # [5, 1000]

    # layer-2: 4-term e4m3 cascade of [w2.T; b2] -> DoubleRow-paired lhsT.
    # Terms t0..t3 at effective scales 1, 2^-4, 2^-12, 2^-16; groups
    # (t0, t1*2^4) and (t2*2^12, t3*2^16) pair with rhs (s, s*2^-4).
    import ml_dtypes
    e4 = ml_dtypes.float8_e4m3

    def q8(a):
        return a.astype(e4).astype(np.float32)

    wb2 = np.concatenate([np.ascontiguousarray(w2.T.astype(np.float32)),
                          b2.astype(np.float32)[None, :]])   # [1001, 1000]
    t0 = q8(wb2)
    r = wb2 - t0
    t1 = q8(r * 2.0 ** 4)
    r = r - t1 * 2.0 ** -4
    t2 = q8(r * 2.0 ** 12)
    r = r - t2 * 2.0 ** -12
    t3 = q8(r * 2.0 ** 16)
    w2p = np.zeros((2, NH, HT + 2, 2, NH * 128), e4)
    terms = ((t0, t1), (t2, t3))
    for g in range(2):
        for k in range(NH):
            for i2 in range(2):
                src = terms[g][i2][k * HT:(k + 1) * HT]      # [125, 1000]
                for h in range(NH):
                    w2p[g, k, :HT, i2, h * 128:h * 128 + HT] = (
                        src[:, h * HT:(h + 1) * HT])
        for i2 in range(2):
            brow = terms[g][i2][H]                           # bias row
            for h in range(NH):
                w2p[g, NH - 1, HT, i2, h * 128:h * 128 + HT] = (
                    brow[h * HT:(h + 1) * HT])

    # layer-3: single 12-bit term; layer-2 spikes arrive as {0,2}, so the
    # weights are halved (exact) and the bias stays plain 24-bit hi/lo
    w3q = _rne12(np.ascontiguousarray(w3.T.astype(np.float32)) * 0.5)
    b3h, b3l = _hilo(b3.astype(np.float32))
    w3r = np.zeros((NH, HT + 2, H3), np.float32)
    for k in range(NH):
        w3r[k, :HT] = w3q[k * HT:(k + 1) * HT]
    w3r[NH - 1, HT] = b3h
    w3r[NH - 1, HT + 1] = b3l

    l1d = np.ascontiguousarray(-np.eye(HT, dtype=np.float32))

    b4h, b4l = _hilo(b4.astype(np.float32))
    w4c = np.zeros((22, H4), np.float32)
    w4c[0:20] = _rne12(w4.T.astype(np.float32))
    w4c[20] = b4h
    w4c[21] = b4l

    in_maps = []
    for c in range(NCORES):
        xc = np.empty((T, XR, B), np.float32)
        xc[:, 0:128, :] = xth[:, 0:128, c * B:(c + 1) * B]
        xc[:, 128:256, :] = xtl[:, 0:128, c * B:(c + 1) * B]
        xc[:, 256, :] = xth[:, 128, c * B:(c + 1) * B]
        xc[:, 257, :] = xtl[:, 128, c * B:(c + 1) * B]
        xc[:, 258, :] = xth[:, 128, c * B:(c + 1) * B]
        xc[:, 259, :] = 1.0
        xc[:, 260, :] = 1.0
        in_maps.append({
            "x_aug": xc, "w1h": w1h, "w1l": w1l, "w1c": w1c, "l1d": l1d,
            "w2p": w2p, "w3r": w3r, "w4c": w4c,
        })
    return in_maps


def _gather(results):
    spk = np.concatenate(
        [np.transpose(r["out_s"], (0, 2, 1)) for r in results], axis=1)
    mem = np.concatenate(
        [np.transpose(r["out_m"], (0, 2, 1)) for r in results], axis=1)
    return spk, mem


def kernel(x, w1, b1, w2, b2, w3, b3, w4, b4, _trace=False, _trace_kwargs=None):
    # accept numpy or jax arrays, any float dtype
    x, w1, b1, w2, b2, w3, b3, w4, b4 = (
        np.asarray(a, dtype=np.float32)
        for a in (x, w1, b1, w2, b2, w3, b3, w4, b4))
    nc = _get_nc()
    in_maps = _prep_inputs(x, w1, b1, w2, b2, w3, b3, w4, b4)
    res = run_bass_kernel_spmd(
        nc, in_maps, core_ids=list(range(NCORES)),
        trace=_trace, **(_trace_kwargs or {}))
    out = _gather(res.results)
    if _trace:
        return out, res
    return out



# revision 57
# speedup vs baseline: 1.1179x; 1.0299x over previous
"""Trainium2 Bass kernel for the AudNet 4-layer LIF spiking network.

Reference computation (per time step t of 81, batch 4096):
    s1, m1 = lif(x_t @ w1.T + b1, m1)     # 129 -> 1000
    s2, m2 = lif(s1 @ w2.T + b2, m2)      # 1000 -> 1000
    s3, m3 = lif(s2 @ w3.T + b3, m3)      # 1000 -> 20
    s4, m4 = lif(s3 @ w4.T + b4, m4)      # 20 -> 10
with lif: reset = (m > 1); m' = 0.95*m + cur - reset; spk = (m' > 1)
Outputs: (s4, m4) per step -> each [81, 4096, 10].

Strategy:
- Data parallel over 8 NeuronCores: 512 batch rows per core; weights
  replicated; no cross-device traffic.
- Hidden-on-partition, batch-on-free layout: weights are the stationary
  lhsT, spikes/x the moving rhs; the 81-step scan needs no transposes.
- Layer 1 (129->1000): fp32r (12-bit) hi/lo cross terms wh@xh + wh@xl +
  wl@xh + a K=5 combo matmul (last feature + 24-bit bias rows), 1 PE
  cycle/column each. The reset subtraction rides the psum group as a -I
  diag matmul so no elementwise pass needs the previous spikes.
- Layer 2 (1000->1000), the dominant cost: 4-term fp8(e4m3) cascade in
  DoubleRow pairs at 0.5 cycles/column and 252 contraction rows per pass
  (verified bit-for-bit on HW; effective weight error ~6e-8, measured
  zero output spike flips). Group 0 lhsT pairs (t0, t1*2^4) multiply rhs
  pairs (s1, s1*2^-4) packed by two ACT converts per tile; group 1 pairs
  (t2*2^12, t3*2^16) reuse the same rhs into a second psum, merged with
  *2^-12 in the membrane update.
- Layers 3/4: single 12-bit fp32r terms (measured flip-free) with 24-bit
  hi/lo bias rows; layer-2 spikes arrive as {0,2} so w3 is halved.
- LIF membrane updates are beta-fused pre-updates: one stt per tile
  m = beta*m + psum (+ the psum's embedded resets/biases), then the spike
  threshold; layer-2 subtracts -0.5*u_old in a second stt. Elementwise
  work is spread so DVE keeps the psum ops, Pool the thresholds/subs, and
  ACT the fp8 packs; per-tile chains pipeline across engines.
- Software pipelining: layer-1 psums + LIF + packs for step t+1 run inside
  iteration t; layer 4 of step t runs inside iteration t+1.
"""

import os
import sys

import numpy as np

for _p in ("/opt/trn_rl_repo", "/root/.axon_site/_ro/trn_rl_repo"):
    if os.path.isdir(_p) and _p not in sys.path:
        sys.path.insert(0, _p)

import concourse.bacc as bacc
import concourse.bass as bass
import concourse.mybir as mybir
import concourse.tile as tile
from concourse.bass_utils import run_bass_kernel_spmd
from concourse.tile_rust import add_dep_helper

# Problem constants (hardcoded; kernel.py must be self-contained).
T = 81          # time steps
F = 129         # input features per step
H = 1000        # hidden units (layers 1, 2)
HT = 125        # hidden tile rows  (H = 8 * 125)
NH = 8          # number of hidden tiles
H3 = 20         # layer-3 units
H4 = 10         # output units
BATCH = 4096
NCORES = 8
B = BATCH // NCORES   # 512 batch rows per core
BETA = 0.95
THRESH = 1.0
XR = 2 * 128 + 5      # x_aug rows: xh[0:128], xl[0:128], 5 combo rows

F32 = mybir.dt.float32
F32R = mybir.dt.float32r
F8 = mybir.dt.float8e4
AOP = mybir.AluOpType
DR = mybir.MatmulPerfMode.DoubleRow
MT = 128              # layer-2 output tile rows, padded to the 128 the
                      # dual-fp8 ldweights ISA check requires (H = 8*125)
HP = NH * MT          # padded layer-2 output columns in the fp8 lhsT


def build_bass():
    # Bacc (not raw Bass): its compile() runs generate_event_semaphores /
    # move_matmul_waits_to_ldweights, required because TRN2 Matmult
    # instructions can carry at most one sync wait.
    nc = bacc.Bacc(trn_type="TRN2", target_bir_lowering=False)

    x_d = nc.dram_tensor("x_aug", [T, XR, B], F32R, kind="ExternalInput")
    w1h_d = nc.dram_tensor("w1h", [128, H], F32R, kind="ExternalInput")
    w1l_d = nc.dram_tensor("w1l", [128, H], F32R, kind="ExternalInput")
    w1c_d = nc.dram_tensor("w1c", [5, H], F32R, kind="ExternalInput")
    # -identity: folds the layer-1 reset subtraction into the psum group,
    # keeping the Pool engine out of the per-step LIF critical chain
    l1d_d = nc.dram_tensor("l1d", [HT, HT], F32R, kind="ExternalInput")
    l1d8_d = nc.dram_tensor("l1d8", [HT, 2, MT], F8, kind="ExternalInput")
    # layer-2 weights as a 4-term fp8(e4m3) cascade in DoubleRow pairs:
    # group 0 lhsT pairs (t0, t1*2^4) with rhs pairs (s, s*2^-4); group 1
    # pairs (t2*2^12, t3*2^16) share the same rhs, psum merged with *2^-12.
    # Effective weight error ~6e-8 (measured zero output spike flips), at
    # 0.5 PE cycles/column and 252 contraction rows per pass.
    w2p_d = nc.dram_tensor("w2p", [2, NH, HT + 2, 2, HP], F8,
                           kind="ExternalInput")
    # layer-3 weights: single 12-bit fp32r term (measured: zero output spike
    # flips end-to-end), 8 matmuls into one [20, B] psum, rhs is s2 directly.
    # Bias stays 24-bit via hi+lo rows 125/126 of the last k-tile (the s2
    # tile carries two ones-rows there).
    w3r_d = nc.dram_tensor("w3r", [NH, HT + 2, H3], F32R, kind="ExternalInput")
    # layer-4 lhsT: rows 0..19 w4 (12-bit, also flip-free), rows 20/21 bias
    # hi/lo riding ones-rows of the s3 tile
    w4c_d = nc.dram_tensor("w4c", [22, H4], F32R, kind="ExternalInput")
    outs_d = nc.dram_tensor("out_s", [T, H4, B], F32, kind="ExternalOutput")
    outm_d = nc.dram_tensor("out_m", [T, H4, B], F32, kind="ExternalOutput")

    with tile.TileContext(nc) as tc:
        with (
            tc.tile_pool(name="pers", bufs=1) as pers,
            tc.tile_pool(name="xpool", bufs=3) as xpool,
            tc.tile_pool(name="ps1", bufs=2, space="PSUM") as ps1,
            tc.tile_pool(name="ps2", bufs=2, space="PSUM") as ps2,
            tc.tile_pool(name="ps3", bufs=1, space="PSUM") as ps3,
            tc.tile_pool(name="ps4", bufs=1, space="PSUM") as ps4,
        ):
            # ---- persistent SBUF tensors ----
            w1h = pers.tile([128, H], F32R, tag="w1h")
            w1l = pers.tile([128, H], F32R, tag="w1l")
            w1c = pers.tile([5, H], F32R, tag="w1c")
            l1d = pers.tile([HT, HT], F32R, tag="l1d")
            l1d8 = pers.tile([HT, 2, MT], F8, tag="l1d8")
            w2p = pers.tile([HT + 2, 2, NH, 2, HP], F8, tag="w2p")
            s1p = pers.tile([HT + 2, 2, NH * B], F8, tag="s1p")
            w3r = pers.tile([HT + 2, NH * H3], F32R, tag="w3r")  # [127, 160]
            w4c = pers.tile([22, H4], F32R, tag="w4c")
            m1 = pers.tile([HT, NH * B], F32, tag="m1")          # [125, 4096]
            m2 = pers.tile([HT, NH * B], F32, tag="m2")
            m3 = pers.tile([H3, B], F32, tag="m3")               # [20, 512]
            m4 = pers.tile([H4, B], F32, tag="m4")               # [10, 512]
            s1 = pers.tile([HT + 1, NH * B], F32R, tag="s1")     # [126, 4096]
            s2 = pers.tile([HT + 2, NH * B], F32R, tag="s2")     # [127, 4096]
            s3 = pers.tile([22, B], F32R, tag="s3")
            s4 = pers.tile([H4, B], F32, tag="s4")

            # fp32 views of the fp32r spike tiles for elementwise consumers
            s1f = s1[:].bitcast(F32)
            s2f = s2[:].bitcast(F32)
            s3f = s3[:].bitcast(F32)

            # ---- weight loads (layer-1 weights + x(0) first: they gate
            # step 0; the bulk w2/w3 transfers follow) ----
            def load_x(t):
                xh = xpool.tile([128, B], F32R, tag="xh", name="xh")
                xl = xpool.tile([128, B], F32R, tag="xl", name="xl")
                xc = xpool.tile([5, B], F32R, tag="xc", name="xc")
                nc.sync.dma_start(xh[:], x_d[t, 0:128, :])
                nc.sync.dma_start(xl[:], x_d[t, 128:256, :])
                nc.sync.dma_start(xc[:], x_d[t, 256:261, :])
                return xh, xl, xc

            w1dmas = []
            for sb, dr in [(w1h, w1h_d), (w1l, w1l_d), (w1c, w1c_d),
                           (l1d, l1d_d), (l1d8, l1d8_d)]:
                w1dmas.append(nc.sync.dma_start(sb[:], dr[:]))
            x0 = load_x(0)
            wdmas = [nc.sync.dma_start(w4c[:], w4c_d[:])]
            for k in range(NH):
                for g in range(2):
                    wdmas.append(
                        nc.sync.dma_start(w2p[:, g, k, :, :], w2p_d[g, k]))
                wdmas.append(nc.sync.dma_start(
                    w3r[:, k * H3:(k + 1) * H3], w3r_d[k]))

            # Matmult instructions can carry at most ONE sync wait in the
            # TRN2 ISA (fp32/fp32r fuse the weight load into the matmul), so
            # have PE nops absorb the weight-DMA waits before any matmul.
            # Layer-1 absorbers go before the prologue; the rest only need to
            # precede the main loop's first layer-2/3/4 matmuls.
            def absorb(dmas):
                nops = []
                for d in dmas:
                    nop = nc.tensor.nop(nofuse=True)
                    add_dep_helper(nop.ins, d.ins, sync=True,
                                   reason="absorb weight-DMA wait on PE")
                    nops.append(nop)
                return nops

            absorbers = absorb(w1dmas)

            # ---- state init ----
            nc.vector.memset(m1[:], 0.0)
            nc.vector.memset(m2[:], 0.0)
            nc.gpsimd.memset(m3[:], 0.0)
            nc.gpsimd.memset(m4[:], 0.0)
            nc.gpsimd.memset(s4[:], 0.0)
            # ones rows feeding the bias fold (k-tile 7 / layer-4 rhs).
            # Engine ops need partition bases in {0,32,64,96}, so memset a
            # wider aligned region; all rows except the last are overwritten
            # by the per-step spike writes before any matmul reads them.
            nc.vector.memset(s1f[96:HT + 1, (NH - 1) * B:], 1.0)
            nc.vector.memset(s2f[96:HT + 2, (NH - 1) * B:], 1.0)
            # pre-update structure reads the previous step's spikes before
            # computing new ones, so spike rows need a zero init AFTER the
            # ones-row memsets above (row 125+ keeps the bias ones)
            nc.vector.memset(s1f[0:HT, :], 0.0)
            nc.vector.memset(s2f[0:HT, :], 0.0)
            nc.gpsimd.memset(s3f[:, :], 1.0)   # rows 20/21 stay as ones rows
            nc.gpsimd.memset(s3f[0:H3, :], 0.0)
            # fp8 rhs pack: row 125 holds the (1, 2^-4) constant pair that
            # multiplies the bias rows of the last k-tile; the ACT pack
            # copies overwrite rows 0..124 every step
            nc.vector.memset(s1p[96:HT + 2, 0, :], 1.0)
            nc.vector.memset(s1p[96:HT + 2, 1, :], 0.0625)
            nc.vector.memset(s1p[0:HT, :, :], 0.0)

            def l1_block(xh, xl, xc):
                """Layer-1 psums + LIF + fp8 packs for one step, per tile.

                Per-tile chain: psum (incl -I*s1_old reset) -> m1 = b*m1+p
                (DVE stt) -> spike (Pool is_gt) -> fp8 packs (ACT), so each
                engine touches a tile once and tiles pipeline."""
                first_mm = None
                for h in range(NH):
                    p1 = ps1.tile([MT, B], F32, tag="p1")
                    c0 = h * HT
                    cols = slice(h * B, (h + 1) * B)
                    # reset via fp8 DoubleRow -I reading the previous
                    # step's pack (odd half x zero lhsT contributes 0)
                    mm = nc.tensor.matmul(p1[:], l1d8[0:HT, :, :],
                                          s1p[0:HT, :, cols],
                                          start=True, stop=False,
                                          perf_mode=DR)
                    if first_mm is None:
                        first_mm = mm
                    nc.tensor.matmul(p1[0:HT, :], w1h[:, c0:c0 + HT], xh[:],
                                     start=False, stop=False)
                    nc.tensor.matmul(p1[0:HT, :], w1h[:, c0:c0 + HT], xl[:],
                                     start=False, stop=False)
                    nc.tensor.matmul(p1[0:HT, :], w1l[:, c0:c0 + HT], xh[:],
                                     start=False, stop=False)
                    nc.tensor.matmul(p1[0:HT, :], w1c[:, c0:c0 + HT], xc[:],
                                     start=False, stop=True)
                    nc.vector.scalar_tensor_tensor(m1[:, cols], m1[:, cols],
                                                   BETA, p1[0:HT, :],
                                                   AOP.mult, AOP.add)
                    nc.gpsimd.tensor_scalar(s1[0:HT, cols], m1[:, cols],
                                            THRESH, None, AOP.is_gt)
                    nc.scalar.copy(s1p[0:HT, 0, cols], s1f[0:HT, cols])
                    nc.scalar.activation(s1p[0:HT, 1, cols], s1f[0:HT, cols],
                                         mybir.ActivationFunctionType.Copy,
                                         scale=0.0625)
                return first_mm

            # ---- prologue: step 0 layer-1 ----
            first_mm = l1_block(*x0)
            for nop in absorbers:
                add_dep_helper(first_mm.ins, nop.ins, sync=False,
                               reason="keep absorbers before first matmul")

            # absorb the remaining weight DMAs before the main loop's
            # layer-2/3/4 matmuls
            late_absorbers = absorb(wdmas)

            def l4_block(t):
                """Layer 4 for step t + LIF + output DMAs."""
                p4 = ps4.tile([H4, B], F32, tag="p4")
                # -I diag carries the reset subtraction (reads s4 of step
                # t-1 before this step's is_gt rewrites it)
                nc.tensor.matmul(p4[:], l1d[0:H4, 0:H4],
                                 s4[:].bitcast(F32R),
                                 start=True, stop=False)
                nc.tensor.matmul(p4[:], w4c[:], s3[:], start=False, stop=True)
                nc.vector.scalar_tensor_tensor(m4[:], m4[:], BETA, p4[:],
                                               AOP.mult, AOP.add)
                nc.sync.dma_start(outm_d[t], m4[:])
                nc.vector.tensor_scalar(s4[:].bitcast(F32R), m4[:], THRESH,
                                        None, AOP.is_gt)
                nc.sync.dma_start(outs_d[t], s4[:])

            # ---- main loop over steps ----
            for i in range(T):
                if i < T - 1:
                    xh, xl, xc = load_x(i + 1)

                # layer 2 of step i: fp8 DoubleRow, two psum scale-groups;
                # each pass consumes 252 contraction rows at 256 PE cycles
                for h in range(NH):
                    c0 = h * MT
                    pA = ps2.tile([MT, B], F32, tag="pA")
                    pB = ps2.tile([MT, B], F32, tag="pB")
                    for g, pX in ((0, pA), (1, pB)):
                        for j in range(NH):
                            k = (h + j) % NH
                            mm2 = nc.tensor.matmul(
                                pX[:],
                                w2p[0:HT + 2, g, k, :, c0:c0 + MT],
                                s1p[0:HT + 2, :, k * B:(k + 1) * B],
                                start=(j == 0),
                                stop=(j == NH - 1),
                                perf_mode=DR)
                            if i == 0 and h == 0 and j == 0 and g == 0:
                                for nop in late_absorbers:
                                    add_dep_helper(
                                        mm2.ins, nop.ins, sync=False,
                                        reason="absorbers before first L2 mm")
                    cols = slice(h * B, (h + 1) * B)
                    # m2 = beta*m2 + psumA + psumB*2^-12 - u_old/2 with
                    # u = 2*(m2>1); layer-3 weights are halved host-side.
                    nc.vector.scalar_tensor_tensor(m2[:, cols], m2[:, cols],
                                                   BETA, pA[0:HT, :],
                                                   AOP.mult, AOP.add)
                    nc.vector.scalar_tensor_tensor(m2[:, cols], pB[0:HT, :],
                                                   2.0 ** -12, m2[:, cols],
                                                   AOP.mult, AOP.add)
                    nc.vector.scalar_tensor_tensor(m2[:, cols],
                                                   s2f[0:HT, cols], -0.5,
                                                   m2[:, cols],
                                                   AOP.mult, AOP.add)
                    # spikes as {0, 2}: exact is_gt (ACT Sign rounds near
                    # the threshold), then *2 in the same instruction; the
                    # halved layer-3 weights and the -0.5 subtract coeff
                    # make u/2 the effective 0/1 spike
                    nc.gpsimd.tensor_scalar(s2[0:HT, cols], m2[:, cols],
                                            THRESH, 2.0, AOP.is_gt, AOP.mult)

                # layer-1 psums + LIF + packs for step i+1
                if i < T - 1:
                    l1_block(xh, xl, xc)

                # layer 4 of step i-1, after the layer-1 block so its wait
                # on the s3 threshold chain is covered by layer-1 PE work
                if i > 0:
                    l4_block(i - 1)

                # layer 3 of step i: single 12-bit fp32r term (weights
                # pre-halved for the +-1 spike convention), 8 matmuls into
                # one [20, B] psum
                p3 = ps3.tile([H3, B], F32, tag="p3")
                # -I diag: reset subtraction reads s3 of step i-1 (after
                # l4_block(i-1) consumed it, before this step's is_gt)
                nc.tensor.matmul(p3[:], l1d[0:H3, 0:H3], s3[0:H3, :],
                                 start=True, stop=False)
                for k in range(NH):
                    kk = HT + 2 if k == NH - 1 else HT
                    nc.tensor.matmul(
                        p3[:],
                        w3r[0:kk, k * H3:(k + 1) * H3],
                        s2[0:kk, k * B:(k + 1) * B],
                        start=False,
                        stop=(k == NH - 1))
                nc.vector.scalar_tensor_tensor(m3[:], m3[:], BETA, p3[:],
                                               AOP.mult, AOP.add)
                nc.vector.tensor_scalar(s3[0:H3, :], m3[:], THRESH, None,
                                        AOP.is_gt)

            # ---- epilogue ----
            l4_block(T - 1)

    nc.compile()
    return nc


_CACHE = {}


def _get_nc():
    if "nc" not in _CACHE:
        _CACHE["nc"] = build_bass()
    return _CACHE["nc"]


def _rne12(a):
    """Round fp32 to 12 significand bits (the fp32r grid), RNE —
    bit-identical to the device's fp32r rounding."""
    drop = np.uint64(12)
    u = np.ascontiguousarray(a, np.float32).view(np.uint32).astype(np.uint64)
    half = np.uint64(1 << 11)
    lsb = (u >> drop) & np.uint64(1)
    u2 = ((u + half - np.uint64(1) + lsb) >> drop << drop)
    return u2.astype(np.uint32).view(np.float32).reshape(a.shape)


def _hilo(a):
    hi = _rne12(a)
    lo = _rne12(np.asarray(a, np.float32) - hi)
    return hi, lo


def _prep_inputs(x, w1, b1, w2, b2, w3, b3, w4, b4):
    x = np.ascontiguousarray(x, np.float32)
    # xs[t, f, b_global]; step t of the reference reads x[:, f*T + t]
    xt = np.ascontiguousarray(
        np.transpose(x.reshape(BATCH, F, T), (2, 1, 0)))   # [T, F, BATCH]
    xth, xtl = _hilo(xt)

    w1T = np.ascontiguousarray(w1.T.astype(np.float32))    # [129, 1000]
    w1h, w1l = _hilo(w1T[:128])
    whL, wlL = _hilo(w1T[128])
    b1h, b1l = _hilo(b1.astype(np.float32))
    w1c = np.stack([whL, whL, wlL, b1h, b1l])              # [5, 1000]

    # layer-2: 4-term e4m3 cascade of [w2.T; b2] -> DoubleRow-paired lhsT.
    # Terms t0..t3 at effective scales 1, 2^-4, 2^-12, 2^-16; groups
    # (t0, t1*2^4) and (t2*2^12, t3*2^16) pair with rhs (s, s*2^-4).
    import ml_dtypes
    e4 = ml_dtypes.float8_e4m3

    def q8(a):
        return a.astype(e4).astype(np.float32)

    wb2 = np.concatenate([np.ascontiguousarray(w2.T.astype(np.float32)),
                          b2.astype(np.float32)[None, :]])   # [1001, 1000]
    t0 = q8(wb2)
    r = wb2 - t0
    t1 = q8(r * 2.0 ** 4)
    r = r - t1 * 2.0 ** -4
    t2 = q8(r * 2.0 ** 12)
    r = r - t2 * 2.0 ** -12
    t3 = q8(r * 2.0 ** 16)
    w2p = np.zeros((2, NH, HT + 2, 2, NH * 128), e4)
    terms = ((t0, t1), (t2, t3))
    for g in range(2):
        for k in range(NH):
            for i2 in range(2):
                src = terms[g][i2][k * HT:(k + 1) * HT]      # [125, 1000]
                for h in range(NH):
                    w2p[g, k, :HT, i2, h * 128:h * 128 + HT] = (
                        src[:, h * HT:(h + 1) * HT])
        for i2 in range(2):
            brow = terms[g][i2][H]                           # bias row
            for h in range(NH):
                w2p[g, NH - 1, HT, i2, h * 128:h * 128 + HT] = (
                    brow[h * HT:(h + 1) * HT])

    # layer-3: single 12-bit term; layer-2 spikes arrive as {0,2}, so the
    # weights are halved (exact) and the bias stays plain 24-bit hi/lo
    w3q = _rne12(np.ascontiguousarray(w3.T.astype(np.float32)) * 0.5)
    b3h, b3l = _hilo(b3.astype(np.float32))
    w3r = np.zeros((NH, HT + 2, H3), np.float32)
    for k in range(NH):
        w3r[k, :HT] = w3q[k * HT:(k + 1) * HT]
    w3r[NH - 1, HT] = b3h
    w3r[NH - 1, HT + 1] = b3l

    l1d = np.ascontiguousarray(-np.eye(HT, dtype=np.float32))
    l1d8 = np.zeros((HT, 2, 128), e4)
    l1d8[:, 0, :HT] = -np.eye(HT, dtype=np.float32)

    b4h, b4l = _hilo(b4.astype(np.float32))
    w4c = np.zeros((22, H4), np.float32)
    w4c[0:20] = _rne12(w4.T.astype(np.float32))
    w4c[20] = b4h
    w4c[21] = b4l

    in_maps = []
    for c in range(NCORES):
        xc = np.empty((T, XR, B), np.float32)
        xc[:, 0:128, :] = xth[:, 0:128, c * B:(c + 1) * B]
        xc[:, 128:256, :] = xtl[:, 0:128, c * B:(c + 1) * B]
        xc[:, 256, :] = xth[:, 128, c * B:(c + 1) * B]
        xc[:, 257, :] = xtl[:, 128, c * B:(c + 1) * B]
        xc[:, 258, :] = xth[:, 128, c * B:(c + 1) * B]
        xc[:, 259, :] = 1.0
        xc[:, 260, :] = 1.0
        in_maps.append({
            "x_aug": xc, "w1h": w1h, "w1l": w1l, "w1c": w1c, "l1d": l1d,
            "l1d8": l1d8,
            "w2p": w2p, "w3r": w3r, "w4c": w4c,
        })
    return in_maps


def _gather(results):
    spk = np.concatenate(
        [np.transpose(r["out_s"], (0, 2, 1)) for r in results], axis=1)
    mem = np.concatenate(
        [np.transpose(r["out_m"], (0, 2, 1)) for r in results], axis=1)
    return spk, mem


def kernel(x, w1, b1, w2, b2, w3, b3, w4, b4, _trace=False, _trace_kwargs=None):
    # accept numpy or jax arrays, any float dtype
    x, w1, b1, w2, b2, w3, b3, w4, b4 = (
        np.asarray(a, dtype=np.float32)
        for a in (x, w1, b1, w2, b2, w3, b3, w4, b4))
    nc = _get_nc()
    in_maps = _prep_inputs(x, w1, b1, w2, b2, w3, b3, w4, b4)
    res = run_bass_kernel_spmd(
        nc, in_maps, core_ids=list(range(NCORES)),
        trace=_trace, **(_trace_kwargs or {}))
    out = _gather(res.results)
    if _trace:
        return out, res
    return out

